# revision 31
# baseline (speedup 1.0000x reference)
"""Two-branch attention kernel for Trainium2 (8 NeuronCores, batch-parallel).

out1 = proj(softmax(q k^T / 8) v),  out2 = proj(softmax(q k2^T / 8) v2)
with q,k,v from x and k2,v2 from x2 (q shared across branches).

Sharding: batch dim (8) -> one batch element per core. No collectives.

Design (per core, transpose-free attention, all-bf16 matmul operands):
  The attention inner loop is ScalarE-exp-paced (2.13us per kj vs 1.7us
  of S+AV matmul), and TRN2's HAM clock-gate re-throttles the PE to
  1.2GHz on every micro-idle, so the whole schedule is built around
  keeping the in-order TensorE stream gapless:
  * prefix: qT/kT group 0 + all v tiles (minimum to start attention);
  * every other QKV group (qT/kT 1-5, k2T, v2) and the projections are
    decomposed into ~0.5us micro-thunks (2-3 matmuls on a dedicated
    1-slot aux PSUM pool) popped between kj iterations so TensorE's
    per-kj work matches the exp pace;
  * AV is split into two 512-wide column passes (po tiles [65,512], one
    PSUM bank) - pass c0 trails S by 2 kj inside the kj loop, pass c1
    runs as a block after it - freeing the PSUM needed for aux;
  * softmax normalize: ones-row -> DRAM bounce -> [16,64] reshape (DVE
    reciprocal is per-lane-bound) -> reciprocal -> bounce out ->
    partition-broadcast read -> multiply straight out of PSUM;
  * wqk stays resident in SBUF so no weight DMAs run mid-attention.
"""
import sys
for _p in ('/opt/trn_rl_repo',):
    if _p not in sys.path:
        sys.path.insert(0, _p)

import numpy as np

# ----------------------------------------------------------------------------
MODE = 'bf16+microweave'  # informational only

B, N, D, H, HD = 8, 1024, 768, 12, 64
SCALE = HD ** -0.5
NDT = D // 128       # 6 dim tiles
NQT = N // 128       # 8 token tiles
P = 128

# ----------------------------------------------------------------------------
# workaround: walrus rejects >2 sem waits on one instruction; TileContext's
# tail drain carries one wait per active logical proc. Split them across
# single-wait SP nops and emit a bare drain.
def _install_tilefix():
    import bass_rust
    import concourse.tile as tile

    def _drain_and_barrier_split(self, tick_clock, wait_clock):
        gc = tick_clock.global_clock
        ticks = [gc[i] for i in range(27)]
        for i, t in enumerate(ticks):
            if t > 0:
                vc = bass_rust.VectorClock(
                    [t if j == i else 0 for j in range(len(ticks))])
                nop = self.nc.sync.nop()
                wait_clock.add_sem_waits(
                    nop.ins, bass_rust.ScopedClock({None: vc}))
        self.nc.sync.drain()
        self.nc.all_engine_barrier()
        assert self.sems is not None
        popped = self.nc._tile_sem_poison_stack.pop()
        assert popped is self._sem_poison
        self.nc.clear_and_free_semaphores(list(self.sems.allocated().values()))
        self.nc.all_engine_barrier()

    tile.TileContext._drain_and_barrier = _drain_and_barrier_split


def _split_multiwaits(nc, max_waits=1):
    """walrus codegen rejects instructions carrying more than `max_waits`
    sync waits; hoist the extras onto same-engine nops placed just before."""
    import bass_rust
    import concourse.mybir as mybir
    cnt = 0
    for bb in nc.main_func.blocks:
        insts = bb.instructions
        i = 0
        while i < len(insts):
            ins = insts[i]
            si = getattr(ins, 'sync_info', None)
            if si is not None and si.on_wait and len(si.on_wait) > max_waits:
                waits = list(si.on_wait)
                extras, keep = waits[:-max_waits], waits[-max_waits:]
                for w in extras:
                    nop = mybir.InstNoOp(name=f"I-swx{cnt}", ins=[], outs=[])
                    cnt += 1
                    nop.engine = ins.engine
                    nop.sync_info = bass_rust.SyncInfo(on_wait=[w],
                                                       on_update=[])
                    insts.insert(i, nop)
                    i += 1
                ins.sync_info = bass_rust.SyncInfo(
                    on_wait=keep, on_update=list(si.on_update))
            i += 1
    return cnt


_built = None


def _build():
    """Build the SPMD bass program once. Returns (nc, n_split_waits)."""
    global _built
    if _built is not None:
        return _built
    _install_tilefix()
    from contextlib import ExitStack
    import concourse.bass as bass
    import concourse.tile as tile
    from concourse import mybir

    dt = mybir.dt
    bdt = dt.bfloat16          # matmul operand dtype throughout

    nc = bass.Bass("TRN2", target_bir_lowering=False, debug=False,
                   num_devices=8)

    # DRAM I/O (per core); x/w tensors come p-major so the big loads are
    # 128 fat contiguous descriptors.
    xt_d = nc.dram_tensor("xt", [P, NDT, N], bdt, kind="ExternalInput")
    x2t_d = nc.dram_tensor("x2t", [P, NDT, N], bdt, kind="ExternalInput")
    wqk_d = nc.dram_tensor("wqk", [P, NDT, 2 * D], bdt,
                           kind="ExternalInput")
    wv_d = nc.dram_tensor("wv", [P, NDT, D], bdt, kind="ExternalInput")
    wp_d = nc.dram_tensor("wp", [P, NDT, D], bdt, kind="ExternalInput")
    bias_d = nc.dram_tensor("bias", [P, D], dt.float32, kind="ExternalInput")
    ones_d = nc.dram_tensor("ones", [P, H, 1], bdt, kind="ExternalInput")
    out_d = nc.dram_tensor("out", [2, N, D], dt.float32,
                           kind="ExternalOutput")

    AUG = HD + 1  # 65: head dim + ones column for row sums

    with tile.TileContext(nc) as tc, ExitStack() as top:
        # PSUM: 16KB/partition total.  8KB S ring + 4KB po ring + 3KB aux.
        pp_s = top.enter_context(tc.tile_pool(name="ps_s", bufs=2,
                                              space="PSUM"))
        pp_o = top.enter_context(tc.tile_pool(name="ps_o", bufs=2,
                                              space="PSUM"))
        pp_x = top.enter_context(tc.tile_pool(name="ps_x", bufs=1,
                                              space="PSUM"))
        dram_rb = top.enter_context(tc.tile_pool(name="dram_rb", bufs=2,
                                                 space="DRAM"))
        persist = top.enter_context(tc.tile_pool(name="persist", bufs=1))
        pool_kv = top.enter_context(tc.tile_pool(name="kv", bufs=1))

        qT = persist.tile([P, NDT, N], bdt, tag="qT")
        wqk_t = persist.tile([P, NDT, 2 * D], bdt, tag="wqk")
        for i in range(NDT):  # per-i chunks: parallel queues + pipelined
            nc.sync.dma_start(out=wqk_t[:, i, :], in_=wqk_d[:, i, :])
        wp_t = persist.tile([P, NDT, D], bdt, tag="wp")
        for i in range(0, NDT, 3):
            nc.sync.dma_start(out=wp_t[:, i:i + 3, :],
                              in_=wp_d[:, i:i + 3, :])
        bias_t = persist.tile([P, D], dt.float32, tag="bias")
        nc.sync.dma_start(out=bias_t, in_=bias_d[:])

        kT = pool_kv.tile([P, NDT, N], bdt, tag="kT")
        kT2 = pool_kv.tile([P, NDT, N], bdt, tag="kT2")
        vaug = pool_kv.tile([P, NQT, H * AUG], bdt, tag="vaug")
        vaug2 = pool_kv.tile([P, NQT, H * AUG], bdt, tag="vaug2")
        xt_t = pool_kv.tile([P, NDT, N], bdt, tag="xt")
        x2t_t = pool_kv.tile([P, NDT, N], bdt, tag="x2t")
        wv_t = pool_kv.tile([P, NDT, D], bdt, tag="wv")

        def load_ones(vaug_t):
            for t in range(NQT):
                nc.sync.dma_start(
                    out=vaug_t[:, t, :].rearrange("p (h e) -> p h e",
                                                  e=AUG)[:, :, HD:AUG],
                    in_=ones_d[:])

        # ---------- macro helpers (prefix use, ScalarE evict) -----------
        def qkv_T_group(xt_tile, wcol0, o, dst_sb, evict):
            ps = pp_s.tile([P, N], dt.float32, tag="S")
            for i in range(NDT):
                for c in range(2):
                    nc.tensor.matmul(
                        ps[:, c * 512:(c + 1) * 512],
                        wqk_t[:, i, wcol0 + o * P: wcol0 + (o + 1) * P],
                        xt_tile[:, i, c * 512:(c + 1) * 512],
                        start=(i == 0), stop=(i == NDT - 1))
            evict(dst_sb[:, o, :], ps[:])

        def v_tile(xt_tile, vaug_t, t, evict):
            ps = pp_s.tile([P, N], dt.float32, tag="S")
            for i in range(NDT):
                for c0, cn in ((0, 512), (512, 256)):
                    nc.tensor.matmul(
                        ps[:, c0:c0 + cn],
                        xt_tile[:, i, t * P:(t + 1) * P],
                        wv_t[:, i, c0:c0 + cn],
                        start=(i == 0), stop=(i == NDT - 1))
            src = ps[:, 0:D].rearrange("p (h e) -> p h e", e=HD)
            dstv = vaug_t[:, t, :].rearrange("p (h e) -> p h e",
                                             e=AUG)[:, :, 0:HD]
            evict(dstv, src)

        # ---------- micro-thunks (attention-time fill, DVE evict) -------
        ev_vec = nc.vector.tensor_copy
        _auxbox = {}

        def qkv_half_micros(xt_tile, wcol0, o, ch, dst_sb):
            """3 micros: 6 accumulating 512-wide matmuls + DVE evict of
            one [128,512] column half of a q/k output group."""
            key = ('qk', id(xt_tile), wcol0, o, ch)

            def mm(i0, n, first, last):
                if first:
                    _auxbox[key] = pp_x.tile([P, 512], dt.float32,
                                             tag="aux", name=f"aux_qk")
                aux = _auxbox[key]
                for i in range(i0, i0 + n):
                    nc.tensor.matmul(
                        aux[:],
                        wqk_t[:, i, wcol0 + o * P: wcol0 + (o + 1) * P],
                        xt_tile[:, i, ch * 512:(ch + 1) * 512],
                        start=(i == 0), stop=(i == NDT - 1),
                        skip_group_check=True)
                if last:
                    ev_vec(dst_sb[:, o, ch * 512:(ch + 1) * 512], aux[:])
                    del _auxbox[key]

            return [lambda: mm(0, 2, True, False),
                    lambda: mm(2, 2, False, False),
                    lambda: mm(4, 2, False, True)]

        def v_tile_micros(xt_tile, vaug_t, t):
            """4 micros: 12 accumulating matmuls + DVE evict of one
            [128 tok, 768] v tile into the ones-augmented buffer."""
            key = ('v', id(xt_tile), t)
            steps = [(i, c0, cn) for i in range(NDT)
                     for c0, cn in ((0, 512), (512, 256))]

            def mm(s0, n, first, last):
                if first:
                    _auxbox[key] = pp_x.tile([P, D], dt.float32,
                                             tag="aux", name=f"aux_v")
                aux = _auxbox[key]
                for i, c0, cn in steps[s0:s0 + n]:
                    nc.tensor.matmul(
                        aux[:, c0:c0 + cn],
                        xt_tile[:, i, t * P:(t + 1) * P],
                        wv_t[:, i, c0:c0 + cn],
                        start=(i == 0), stop=(i == NDT - 1),
                        skip_group_check=True)
                if last:
                    src = aux[:, 0:D].rearrange("p (h e) -> p h e", e=HD)
                    dstv = vaug_t[:, t, :].rearrange(
                        "p (h e) -> p h e", e=AUG)[:, :, 0:HD]
                    ev_vec(dstv, src)
                    del _auxbox[key]

            return [lambda: mm(0, 3, True, False),
                    lambda: mm(3, 3, False, False),
                    lambda: mm(6, 3, False, False),
                    lambda: mm(9, 3, False, True)]

        pool_res = top.enter_context(tc.tile_pool(name="res", bufs=2))

        def proj_micros(ot_t, br, qi, pool=pp_x):
            """4 micros: 12 accumulating matmuls + bias add + DMA out of
            one [128 tok, 768] projection output tile."""
            key = ('p', br, qi)
            steps = [(g, c0, cn) for g in range(NDT)
                     for c0, cn in ((0, 512), (512, 256))]

            def mm(s0, n, first, last):
                if first:
                    _auxbox[key] = pool.tile([P, D], dt.float32,
                                             tag="aux" if pool is pp_x
                                             else "S", name=f"aux_p")
                aux = _auxbox[key]
                for g, c0, cn in steps[s0:s0 + n]:
                    nc.tensor.matmul(
                        aux[:, c0:c0 + cn],
                        ot_t[:, g, qi * P:(qi + 1) * P],
                        wp_t[:, g, c0:c0 + cn],
                        start=(g == 0), stop=(g == NDT - 1),
                        skip_group_check=True)
                if last:
                    res = pool_res.tile([P, D], dt.float32, tag="res")
                    nc.vector.tensor_add(res[:], aux[:], bias_t[:])
                    nc.sync.dma_start(
                        out=out_d[br, qi * P:(qi + 1) * P, :], in_=res[:])
                    del _auxbox[key]

            return [lambda: mm(0, 3, True, False),
                    lambda: mm(3, 3, False, False),
                    lambda: mm(6, 3, False, False),
                    lambda: mm(9, 3, False, True)]

        # ============ phase A prefix: minimum to start attention ========
        for i in range(NDT):
            nc.sync.dma_start(out=xt_t[:, i, :], in_=xt_d[:, i, :])
        for i in range(0, NDT, 2):
            nc.sync.dma_start(out=wv_t[:, i:i + 2, :],
                              in_=wv_d[:, i:i + 2, :])
        for i in range(0, NDT, 2):
            nc.sync.dma_start(out=x2t_t[:, i:i + 2, :],
                              in_=x2t_d[:, i:i + 2, :])
        qkv_T_group(xt_t, 0, 0, qT, nc.scalar.copy)
        qkv_T_group(xt_t, D, 0, kT, nc.scalar.copy)
        load_ones(vaug)
        load_ones(vaug2)
        for t in range(NQT):
            v_tile(xt_t, vaug, t, nc.scalar.copy)

        # ================= phase B: attention + proj ====================
        pool_pt = top.enter_context(tc.tile_pool(name="pt", bufs=2))
        pool_ot = top.enter_context(tc.tile_pool(name="ot", bufs=2))
        pool_sm = top.enter_context(tc.tile_pool(name="sm", bufs=2))
        pool_osb = top.enter_context(tc.tile_pool(name="osb", bufs=2))

        def normalize(po_c, ot, g, ch):
            """o^T[:, g, ch] /= rowsum via the ones rows of po_c[0..1].
            po is evicted to SBUF first (cheap) so the PSUM slot frees
            immediately; the reciprocal/broadcast DMA chain then runs
            fully async off the SBUF copy."""
            CW = 512
            osb = [pool_osb.tile([AUG, CW], bdt, tag="osb",
                                 name=f"osb{g}_{ch}_{hh}")
                   for hh in range(2)]
            for hh in range(2):
                nc.vector.tensor_copy(osb[hh][:], po_c[hh][:])
            rb1 = dram_rb.tile([2, CW], bdt, tag="rb1")
            for hh in range(2):
                nc.sync.dma_start(out=rb1[hh:hh + 1, :],
                                  in_=osb[hh][HD:HD + 1, :])
            rgs = pool_sm.tile([16, HD], bdt, tag="rgs")
            nc.sync.dma_start(
                out=rgs[:], in_=rb1[:].rearrange("h (p e) -> (h p) e",
                                                 e=HD))
            rr = pool_sm.tile([16, HD], dt.float32, tag="rr")
            nc.vector.reciprocal(rr[:], rgs[:])
            rb2 = dram_rb.tile([2, CW], dt.float32, tag="rb2")
            nc.sync.dma_start(
                out=rb2[:].rearrange("h (p e) -> (h p) e", e=HD), in_=rr[:])
            for hh in range(2):
                rb = pool_sm.tile([HD, CW], dt.float32, tag="rb")
                nc.sync.dma_start(
                    out=rb[:], in_=rb2[hh, :].partition_broadcast(HD))
                nc.vector.tensor_mul(
                    ot[hh * HD:(hh + 1) * HD, g,
                       ch * 512:(ch + 1) * 512],
                    osb[hh][0:HD, :], rb[:])

        def attention(kT_t, vaug_t, br, micros, hold=0):
            """Exp-paced kj loop with AV pass c0 trailing S by 2 kj and
            micro-thunks popped per kj; AV pass c1 + second normalize run
            as the per-g tail block."""
            HQ = NQT // 2
            n0 = len(micros) - hold
            nslot = NDT * NQT
            done = [0]

            def pace(slot):
                want = n0 * (slot + 1) // nslot
                while done[0] < want and len(micros) > hold:
                    micros.pop(0)()
                    done[0] += 1

            ot = pool_ot.tile([P, NDT, N], bdt, tag="ot")
            for g in range(NDT):
                po0 = [pp_o.tile([AUG, 512], dt.float32, tag="O",
                                 name=f"po0_{br}_{g}_{hh}")
                       for hh in range(2)]
                pth = {}

                def emit_av(po_c, ch, kj):
                    p = pth[kj // HQ]
                    for hh in range(2):
                        h = 2 * g + hh
                        nc.tensor.matmul(
                            po_c[hh][:],
                            vaug_t[:, kj, h * AUG:(h + 1) * AUG],
                            p[:, hh, kj % HQ, ch * 512:(ch + 1) * 512],
                            start=(kj == 0), stop=(kj == NQT - 1),
                            skip_group_check=True)

                for kj in range(NQT):
                    if kj % HQ == 0:
                        pth[kj // HQ] = pool_pt.tile(
                            [P, 2, HQ, N], bdt, tag="pt",
                            name=f"pth{br}_{g}_{kj // HQ}")
                    kjl = kj % HQ
                    pse = pp_s.tile([P, N], dt.float32, tag="S")
                    pso = pp_s.tile([P, N], dt.float32, tag="S")
                    for c in range(2):
                        nc.tensor.matmul(
                            pse[:, c * 512:(c + 1) * 512],
                            kT_t[0:HD, g, kj * P:(kj + 1) * P],
                            qT[0:HD, g, c * 512:(c + 1) * 512],
                            start=True, stop=True)
                        nc.tensor.matmul(
                            pso[:, c * 512:(c + 1) * 512],
                            kT_t[HD:P, g, kj * P:(kj + 1) * P],
                            qT[HD:P, g, c * 512:(c + 1) * 512],
                            start=True, stop=True)
                    nc.scalar.activation(
                        pth[kj // HQ][:, 0, kjl, :], pse[:],
                        mybir.ActivationFunctionType.Exp, scale=SCALE)
                    nc.scalar.activation(
                        pth[kj // HQ][:, 1, kjl, :], pso[:],
                        mybir.ActivationFunctionType.Exp, scale=SCALE)
                    if kj >= 2:
                        emit_av(po0, 0, kj - 2)
                    pace(g * NQT + kj)
                emit_av(po0, 0, NQT - 2)
                emit_av(po0, 0, NQT - 1)
                normalize(po0, ot, g, 0)
                po1 = [pp_o.tile([AUG, 512], dt.float32, tag="O",
                                 name=f"po1_{br}_{g}_{hh}")
                       for hh in range(2)]
                for kj in range(NQT):
                    emit_av(po1, 1, kj)
                normalize(po1, ot, g, 1)
            while len(micros) > hold:
                micros.pop(0)()
            return ot

        # branch 0: fill with qT/kT groups 1-5 (needed one g ahead),
        # x2's first k2T group, and all v2 tiles (needed by br1 start).
        micros = []
        for o in range(1, NDT):
            for ch in range(2):
                micros += qkv_half_micros(xt_t, 0, o, ch, qT)
            for ch in range(2):
                micros += qkv_half_micros(xt_t, D, o, ch, kT)
        for ch in range(2):
            micros += qkv_half_micros(x2t_t, D, 0, ch, kT2)
        for t in range(NQT):
            micros += v_tile_micros(x2t_t, vaug2, t)
        ot0 = attention(kT, vaug, 0, micros)

        # branch 1: fill with x2's remaining k2T groups (one g ahead) and
        # branch-0 proj; hold the last 2 projs for the tail.
        micros = []
        for o in range(1, NDT):
            for ch in range(2):
                micros += qkv_half_micros(x2t_t, D, o, ch, kT2)
        for qi in range(NQT):
            micros += proj_micros(ot0, 0, qi)
        ot1 = attention(kT2, vaug2, 1, micros, hold=16)

        # tail: the held branch-0 projs interleave with branch-1 projs
        # (helds don't depend on ot1's last normalize chain, covering its
        # latency).  pp_x groups must run whole (bufs=1 ring) — never
        # split a group across another's; pp_s groups go between.
        tail = list(micros)  # 4 held proj groups (pp_x), 4 micros each
        order = [[tail.pop(0) for _ in range(4)]]
        for qi in range(NQT):
            order.append(proj_micros(ot1, 1, qi,
                                     pool=(pp_s if qi % 2 == 0 else pp_x)))
            if qi < 3 and tail:
                order.append([tail.pop(0) for _ in range(4)])
        for grp in order:
            for m in grp:
                m()

    n = _split_multiwaits(nc)
    _built = (nc, n)
    return _built


def _host_prep(x, x2, qkv_w, proj_w, proj_b):
    """-> list of 8 per-core input maps; matmul operands in bfloat16,
    x^T/w tensors p-major ([128, i, cols]) for fat DMA descriptors."""
    import ml_dtypes
    bf16 = ml_dtypes.bfloat16
    b16 = lambda a: np.ascontiguousarray(np.asarray(a), dtype=bf16)

    def pmaj(m):  # [768, cols] -> [128, 6, cols]
        return np.ascontiguousarray(
            np.asarray(m).reshape(NDT, P, -1).transpose(1, 0, 2))

    xt = np.transpose(np.asarray(x), (0, 2, 1))      # [B, 768, 1024]
    x2t = np.transpose(np.asarray(x2), (0, 2, 1))
    wqk = b16(pmaj(np.asarray(qkv_w)[:2 * D].T))     # [128, 6, 1536]
    wv = b16(pmaj(np.asarray(qkv_w)[2 * D:].T))      # [128, 6, 768]
    wp = b16(pmaj(np.asarray(proj_w).T))             # [128, 6, 768]
    bias = np.broadcast_to(np.asarray(proj_b, dtype=np.float32),
                           (P, D)).copy()
    ones = np.ones((P, H, 1), dtype=bf16)
    maps = []
    for c in range(B):
        maps.append({
            "xt": b16(pmaj(xt[c])),
            "x2t": b16(pmaj(x2t[c])),
            "wqk": wqk, "wv": wv, "wp": wp, "bias": bias,
            "ones": ones,
        })
    return maps


def kernel(x, x2, qkv_w, proj_w, proj_b, trace=False, tmpdir=None):
    nc, _ = _build()
    from concourse.bass_utils import run_bass_kernel_spmd
    in_maps = _host_prep(x, x2, qkv_w, proj_w, proj_b)
    res = run_bass_kernel_spmd(nc, in_maps, list(range(B)), trace=trace,
                               tmpdir=tmpdir)
    kernel.last_exec_time_ns = res.exec_time_ns
    out = np.stack([res.results[c]["out"] for c in range(B)])  # [B,2,N,D]
    out1 = np.ascontiguousarray(out[:, 0])
    out2 = np.ascontiguousarray(out[:, 1])
    return (out1, out2)


kernel.last_exec_time_ns = None


# revision 34
# speedup vs baseline: 1.0281x; 1.0281x over previous
"""Two-branch attention kernel for Trainium2 (8 NeuronCores, batch-parallel).

out1 = proj(softmax(q k^T / 8) v),  out2 = proj(softmax(q k2^T / 8) v2)
with q,k,v from x and k2,v2 from x2 (q shared across branches).

Sharding: batch dim (8) -> one batch element per core. No collectives.

Design (per core, transpose-free attention, all-bf16 matmul operands):
  The attention inner loop is ScalarE-exp-paced (2.13us per kj vs 1.7us
  of S+AV matmul), and TRN2's HAM clock-gate re-throttles the PE to
  1.2GHz on every micro-idle, so the whole schedule is built around
  keeping the in-order TensorE stream gapless:
  * prefix: qT/kT group 0 + all v tiles (minimum to start attention);
  * every other QKV group (qT/kT 1-5, k2T, v2) and the projections are
    decomposed into ~0.5us micro-thunks (2-3 matmuls on a dedicated
    1-slot aux PSUM pool) popped between kj iterations so TensorE's
    per-kj work matches the exp pace;
  * AV is split into two 512-wide column passes (po tiles [65,512], one
    PSUM bank) - pass c0 trails S by 2 kj inside the kj loop, pass c1
    runs as a block after it - freeing the PSUM needed for aux;
  * softmax normalize: ones-row -> DRAM bounce -> [16,64] reshape (DVE
    reciprocal is per-lane-bound) -> reciprocal -> bounce out ->
    partition-broadcast read -> multiply straight out of PSUM;
  * wqk stays resident in SBUF so no weight DMAs run mid-attention.
"""
import sys
for _p in ('/opt/trn_rl_repo',):
    if _p not in sys.path:
        sys.path.insert(0, _p)

import numpy as np

# ----------------------------------------------------------------------------
MODE = 'bf16+microweave'  # informational only

B, N, D, H, HD = 8, 1024, 768, 12, 64
SCALE = HD ** -0.5
NDT = D // 128       # 6 dim tiles
NQT = N // 128       # 8 token tiles
P = 128

# ----------------------------------------------------------------------------
# workaround: walrus rejects >2 sem waits on one instruction; TileContext's
# tail drain carries one wait per active logical proc. Split them across
# single-wait SP nops and emit a bare drain.
def _install_tilefix():
    import bass_rust
    import concourse.tile as tile

    def _drain_and_barrier_split(self, tick_clock, wait_clock):
        gc = tick_clock.global_clock
        ticks = [gc[i] for i in range(27)]
        for i, t in enumerate(ticks):
            if t > 0:
                vc = bass_rust.VectorClock(
                    [t if j == i else 0 for j in range(len(ticks))])
                nop = self.nc.sync.nop()
                wait_clock.add_sem_waits(
                    nop.ins, bass_rust.ScopedClock({None: vc}))
        self.nc.sync.drain()
        self.nc.all_engine_barrier()
        assert self.sems is not None
        popped = self.nc._tile_sem_poison_stack.pop()
        assert popped is self._sem_poison
        self.nc.clear_and_free_semaphores(list(self.sems.allocated().values()))
        self.nc.all_engine_barrier()

    tile.TileContext._drain_and_barrier = _drain_and_barrier_split


def _split_multiwaits(nc, max_waits=1):
    """walrus codegen rejects instructions carrying more than `max_waits`
    sync waits; hoist the extras onto same-engine nops placed just before."""
    import bass_rust
    import concourse.mybir as mybir
    cnt = 0
    for bb in nc.main_func.blocks:
        insts = bb.instructions
        i = 0
        while i < len(insts):
            ins = insts[i]
            si = getattr(ins, 'sync_info', None)
            if si is not None and si.on_wait and len(si.on_wait) > max_waits:
                waits = list(si.on_wait)
                extras, keep = waits[:-max_waits], waits[-max_waits:]
                for w in extras:
                    nop = mybir.InstNoOp(name=f"I-swx{cnt}", ins=[], outs=[])
                    cnt += 1
                    nop.engine = ins.engine
                    nop.sync_info = bass_rust.SyncInfo(on_wait=[w],
                                                       on_update=[])
                    insts.insert(i, nop)
                    i += 1
                ins.sync_info = bass_rust.SyncInfo(
                    on_wait=keep, on_update=list(si.on_update))
            i += 1
    return cnt


_built = None


def _build():
    """Build the SPMD bass program once. Returns (nc, n_split_waits)."""
    global _built
    if _built is not None:
        return _built
    _install_tilefix()
    from contextlib import ExitStack
    import concourse.bass as bass
    import concourse.tile as tile
    from concourse import mybir

    dt = mybir.dt
    bdt = dt.bfloat16          # matmul operand dtype throughout

    nc = bass.Bass("TRN2", target_bir_lowering=False, debug=False,
                   num_devices=8)

    # DRAM I/O (per core); x/w tensors come p-major so the big loads are
    # 128 fat contiguous descriptors.
    xt_d = nc.dram_tensor("xt", [P, NDT, N], bdt, kind="ExternalInput")
    x2t_d = nc.dram_tensor("x2t", [P, NDT, N], bdt, kind="ExternalInput")
    wqk_d = nc.dram_tensor("wqk", [P, NDT, 2 * D], bdt,
                           kind="ExternalInput")
    wv_d = nc.dram_tensor("wv", [P, NDT, D], bdt, kind="ExternalInput")
    wp_d = nc.dram_tensor("wp", [P, NDT, D], bdt, kind="ExternalInput")
    bias_d = nc.dram_tensor("bias", [P, D], dt.float32, kind="ExternalInput")
    ones_d = nc.dram_tensor("ones", [P, H, 1], bdt, kind="ExternalInput")
    out_d = nc.dram_tensor("out", [2, N, D], dt.float32,
                           kind="ExternalOutput")

    AUG = HD + 1  # 65: head dim + ones column for row sums

    with tile.TileContext(nc) as tc, ExitStack() as top:
        # PSUM: 16KB/partition total.  8KB S ring + 4KB po ring + 3KB aux.
        pp_s = top.enter_context(tc.tile_pool(name="ps_s", bufs=2,
                                              space="PSUM"))
        pp_o = top.enter_context(tc.tile_pool(name="ps_o", bufs=2,
                                              space="PSUM"))
        pp_x = top.enter_context(tc.tile_pool(name="ps_x", bufs=1,
                                              space="PSUM"))
        dram_rb = top.enter_context(tc.tile_pool(name="dram_rb", bufs=2,
                                                 space="DRAM"))
        persist = top.enter_context(tc.tile_pool(name="persist", bufs=1))
        pool_kv = top.enter_context(tc.tile_pool(name="kv", bufs=1))

        qT = persist.tile([P, NDT, N], bdt, tag="qT")
        wqk_t = persist.tile([P, NDT, 2 * D], bdt, tag="wqk")
        for i in range(NDT):  # per-i chunks: parallel queues + pipelined
            nc.sync.dma_start(out=wqk_t[:, i, :], in_=wqk_d[:, i, :])
        wp_t = persist.tile([P, NDT, D], bdt, tag="wp")
        bias_t = persist.tile([P, D], dt.float32, tag="bias")
        nc.sync.dma_start(out=bias_t, in_=bias_d[:])

        kT = pool_kv.tile([P, NDT, N], bdt, tag="kT")
        kT2 = pool_kv.tile([P, NDT, N], bdt, tag="kT2")
        vaug = pool_kv.tile([P, NQT, H * AUG], bdt, tag="vaug")
        vaug2 = pool_kv.tile([P, NQT, H * AUG], bdt, tag="vaug2")
        xt_t = pool_kv.tile([P, NDT, N], bdt, tag="xt")
        x2t_t = pool_kv.tile([P, NDT, N], bdt, tag="x2t")
        wv_t = pool_kv.tile([P, NDT, D], bdt, tag="wv")

        def load_ones(vaug_t):
            for t in range(NQT):
                nc.sync.dma_start(
                    out=vaug_t[:, t, :].rearrange("p (h e) -> p h e",
                                                  e=AUG)[:, :, HD:AUG],
                    in_=ones_d[:])

        # ---------- macro helpers (prefix use, ScalarE evict) -----------
        def qkv_T_group(xt_tile, wcol0, o, dst_sb, evict):
            ps = pp_s.tile([P, N], dt.float32, tag="S")
            for i in range(NDT):
                for c in range(2):
                    nc.tensor.matmul(
                        ps[:, c * 512:(c + 1) * 512],
                        wqk_t[:, i, wcol0 + o * P: wcol0 + (o + 1) * P],
                        xt_tile[:, i, c * 512:(c + 1) * 512],
                        start=(i == 0), stop=(i == NDT - 1))
            evict(dst_sb[:, o, :], ps[:])

        def v_tile(xt_tile, vaug_t, t, evict):
            ps = pp_s.tile([P, N], dt.float32, tag="S")
            for i in range(NDT):
                for c0, cn in ((0, 512), (512, 256)):
                    nc.tensor.matmul(
                        ps[:, c0:c0 + cn],
                        xt_tile[:, i, t * P:(t + 1) * P],
                        wv_t[:, i, c0:c0 + cn],
                        start=(i == 0), stop=(i == NDT - 1))
            src = ps[:, 0:D].rearrange("p (h e) -> p h e", e=HD)
            dstv = vaug_t[:, t, :].rearrange("p (h e) -> p h e",
                                             e=AUG)[:, :, 0:HD]
            evict(dstv, src)

        # ---------- micro-thunks (attention-time fill, DVE evict) -------
        ev_vec = nc.vector.tensor_copy
        _auxbox = {}

        def qkv_half_micros(xt_tile, wcol0, o, ch, dst_sb):
            """3 micros: 6 accumulating 512-wide matmuls + DVE evict of
            one [128,512] column half of a q/k output group."""
            key = ('qk', id(xt_tile), wcol0, o, ch)

            def mm(i0, n, first, last):
                if first:
                    _auxbox[key] = pp_x.tile([P, 512], dt.float32,
                                             tag="aux", name=f"aux_qk")
                aux = _auxbox[key]
                for i in range(i0, i0 + n):
                    nc.tensor.matmul(
                        aux[:],
                        wqk_t[:, i, wcol0 + o * P: wcol0 + (o + 1) * P],
                        xt_tile[:, i, ch * 512:(ch + 1) * 512],
                        start=(i == 0), stop=(i == NDT - 1),
                        skip_group_check=True)
                if last:
                    ev_vec(dst_sb[:, o, ch * 512:(ch + 1) * 512], aux[:])
                    del _auxbox[key]

            return [lambda: mm(0, 2, True, False),
                    lambda: mm(2, 2, False, False),
                    lambda: mm(4, 2, False, True)]

        def v_tile_micros(xt_tile, vaug_t, t):
            """4 micros: 12 accumulating matmuls + DVE evict of one
            [128 tok, 768] v tile into the ones-augmented buffer."""
            key = ('v', id(xt_tile), t)
            steps = [(i, c0, cn) for i in range(NDT)
                     for c0, cn in ((0, 512), (512, 256))]

            def mm(s0, n, first, last):
                if first:
                    _auxbox[key] = pp_x.tile([P, D], dt.float32,
                                             tag="aux", name=f"aux_v")
                aux = _auxbox[key]
                for i, c0, cn in steps[s0:s0 + n]:
                    nc.tensor.matmul(
                        aux[:, c0:c0 + cn],
                        xt_tile[:, i, t * P:(t + 1) * P],
                        wv_t[:, i, c0:c0 + cn],
                        start=(i == 0), stop=(i == NDT - 1),
                        skip_group_check=True)
                if last:
                    src = aux[:, 0:D].rearrange("p (h e) -> p h e", e=HD)
                    dstv = vaug_t[:, t, :].rearrange(
                        "p (h e) -> p h e", e=AUG)[:, :, 0:HD]
                    ev_vec(dstv, src)
                    del _auxbox[key]

            return [lambda: mm(0, 3, True, False),
                    lambda: mm(3, 3, False, False),
                    lambda: mm(6, 3, False, False),
                    lambda: mm(9, 3, False, True)]

        pool_res = top.enter_context(tc.tile_pool(name="res", bufs=2))

        def proj_micros(ot_t, br, qi, pool=pp_x):
            """4 micros: 12 accumulating matmuls + bias add + DMA out of
            one [128 tok, 768] projection output tile."""
            key = ('p', br, qi)
            steps = [(g, c0, cn) for g in range(NDT)
                     for c0, cn in ((0, 512), (512, 256))]

            def mm(s0, n, first, last):
                if first:
                    _auxbox[key] = pool.tile([P, D], dt.float32,
                                             tag="aux" if pool is pp_x
                                             else "S", name=f"aux_p")
                aux = _auxbox[key]
                for g, c0, cn in steps[s0:s0 + n]:
                    nc.tensor.matmul(
                        aux[:, c0:c0 + cn],
                        ot_t[:, g, qi * P:(qi + 1) * P],
                        wp_t[:, g, c0:c0 + cn],
                        start=(g == 0), stop=(g == NDT - 1),
                        skip_group_check=True)
                if last:
                    res = pool_res.tile([P, D], dt.float32, tag="res")
                    nc.vector.tensor_add(res[:], aux[:], bias_t[:])
                    nc.sync.dma_start(
                        out=out_d[br, qi * P:(qi + 1) * P, :], in_=res[:])
                    del _auxbox[key]

            return [lambda: mm(0, 3, True, False),
                    lambda: mm(3, 3, False, False),
                    lambda: mm(6, 3, False, False),
                    lambda: mm(9, 3, False, True)]

        # ============ phase A prefix: minimum to start attention ========
        for i in range(NDT):
            nc.sync.dma_start(out=xt_t[:, i, :], in_=xt_d[:, i, :])
        for i in range(0, NDT, 2):
            nc.sync.dma_start(out=wv_t[:, i:i + 2, :],
                              in_=wv_d[:, i:i + 2, :])
        qkv_T_group(xt_t, 0, 0, qT, nc.scalar.copy)
        qkv_T_group(xt_t, D, 0, kT, nc.scalar.copy)
        load_ones(vaug)
        load_ones(vaug2)
        for t in range(NQT):
            v_tile(xt_t, vaug, t, nc.scalar.copy)
        # x2^T and W_proj aren't touched until well into attention;
        # emitting their loads here keeps them off the startup's
        # bandwidth-bound critical path.
        for i in range(0, NDT, 2):
            nc.sync.dma_start(out=x2t_t[:, i:i + 2, :],
                              in_=x2t_d[:, i:i + 2, :])
        for i in range(0, NDT, 3):
            nc.sync.dma_start(out=wp_t[:, i:i + 3, :],
                              in_=wp_d[:, i:i + 3, :])

        # ================= phase B: attention + proj ====================
        pool_pt = top.enter_context(tc.tile_pool(name="pt", bufs=2))
        pool_ot = top.enter_context(tc.tile_pool(name="ot", bufs=2))
        pool_sm = top.enter_context(tc.tile_pool(name="sm", bufs=2))
        pool_osb = top.enter_context(tc.tile_pool(name="osb", bufs=2))

        def normalize(po_c, ot, g, ch):
            """o^T[:, g, ch] /= rowsum via the ones rows of po_c[0..1].
            po is evicted to SBUF first (cheap) so the PSUM slot frees
            immediately; the reciprocal/broadcast DMA chain then runs
            fully async off the SBUF copy."""
            CW = 512
            osb = [pool_osb.tile([AUG, CW], bdt, tag="osb",
                                 name=f"osb{g}_{ch}_{hh}")
                   for hh in range(2)]
            for hh in range(2):
                nc.vector.tensor_copy(osb[hh][:], po_c[hh][:])
            rb1 = dram_rb.tile([2, CW], bdt, tag="rb1")
            for hh in range(2):
                nc.sync.dma_start(out=rb1[hh:hh + 1, :],
                                  in_=osb[hh][HD:HD + 1, :])
            rgs = pool_sm.tile([16, HD], bdt, tag="rgs")
            nc.sync.dma_start(
                out=rgs[:], in_=rb1[:].rearrange("h (p e) -> (h p) e",
                                                 e=HD))
            rr = pool_sm.tile([16, HD], dt.float32, tag="rr")
            nc.vector.reciprocal(rr[:], rgs[:])
            rb2 = dram_rb.tile([2, CW], dt.float32, tag="rb2")
            nc.sync.dma_start(
                out=rb2[:].rearrange("h (p e) -> (h p) e", e=HD), in_=rr[:])
            for hh in range(2):
                rb = pool_sm.tile([HD, CW], dt.float32, tag="rb")
                nc.sync.dma_start(
                    out=rb[:], in_=rb2[hh, :].partition_broadcast(HD))
                nc.vector.tensor_mul(
                    ot[hh * HD:(hh + 1) * HD, g,
                       ch * 512:(ch + 1) * 512],
                    osb[hh][0:HD, :], rb[:])

        def attention(kT_t, vaug_t, br, micros, hold=0):
            """Exp-paced kj loop with AV pass c0 trailing S by 2 kj and
            micro-thunks popped per kj; AV pass c1 + second normalize run
            as the per-g tail block."""
            HQ = NQT // 2
            n0 = len(micros) - hold
            nslot = NDT * NQT
            done = [0]

            def pace(slot):
                want = n0 * (slot + 1) // nslot
                while done[0] < want and len(micros) > hold:
                    micros.pop(0)()
                    done[0] += 1

            ot = pool_ot.tile([P, NDT, N], bdt, tag="ot")
            for g in range(NDT):
                po0 = [pp_o.tile([AUG, 512], dt.float32, tag="O",
                                 name=f"po0_{br}_{g}_{hh}")
                       for hh in range(2)]
                pth = {}

                def emit_av(po_c, ch, kj):
                    p = pth[kj // HQ]
                    for hh in range(2):
                        h = 2 * g + hh
                        nc.tensor.matmul(
                            po_c[hh][:],
                            vaug_t[:, kj, h * AUG:(h + 1) * AUG],
                            p[:, hh, kj % HQ, ch * 512:(ch + 1) * 512],
                            start=(kj == 0), stop=(kj == NQT - 1),
                            skip_group_check=True)

                for kj in range(NQT):
                    if kj % HQ == 0:
                        pth[kj // HQ] = pool_pt.tile(
                            [P, 2, HQ, N], bdt, tag="pt",
                            name=f"pth{br}_{g}_{kj // HQ}")
                    kjl = kj % HQ
                    pse = pp_s.tile([P, N], dt.float32, tag="S")
                    pso = pp_s.tile([P, N], dt.float32, tag="S")
                    for c in range(2):
                        nc.tensor.matmul(
                            pse[:, c * 512:(c + 1) * 512],
                            kT_t[0:HD, g, kj * P:(kj + 1) * P],
                            qT[0:HD, g, c * 512:(c + 1) * 512],
                            start=True, stop=True)
                        nc.tensor.matmul(
                            pso[:, c * 512:(c + 1) * 512],
                            kT_t[HD:P, g, kj * P:(kj + 1) * P],
                            qT[HD:P, g, c * 512:(c + 1) * 512],
                            start=True, stop=True)
                    nc.scalar.activation(
                        pth[kj // HQ][:, 0, kjl, :], pse[:],
                        mybir.ActivationFunctionType.Exp, scale=SCALE)
                    nc.scalar.activation(
                        pth[kj // HQ][:, 1, kjl, :], pso[:],
                        mybir.ActivationFunctionType.Exp, scale=SCALE)
                    if kj >= 2:
                        emit_av(po0, 0, kj - 2)
                    pace(g * NQT + kj)
                emit_av(po0, 0, NQT - 2)
                emit_av(po0, 0, NQT - 1)
                normalize(po0, ot, g, 0)
                po1 = [pp_o.tile([AUG, 512], dt.float32, tag="O",
                                 name=f"po1_{br}_{g}_{hh}")
                       for hh in range(2)]
                for kj in range(NQT):
                    emit_av(po1, 1, kj)
                normalize(po1, ot, g, 1)
            while len(micros) > hold:
                micros.pop(0)()
            return ot

        # branch 0: fill with qT/kT groups 1-5 (needed one g ahead),
        # x2's first k2T group, and all v2 tiles (needed by br1 start).
        micros = []
        for o in range(1, NDT):
            for ch in range(2):
                micros += qkv_half_micros(xt_t, 0, o, ch, qT)
            for ch in range(2):
                micros += qkv_half_micros(xt_t, D, o, ch, kT)
        for ch in range(2):
            micros += qkv_half_micros(x2t_t, D, 0, ch, kT2)
        for t in range(NQT):
            micros += v_tile_micros(x2t_t, vaug2, t)
        ot0 = attention(kT, vaug, 0, micros)

        # branch 1: fill with x2's remaining k2T groups (one g ahead) and
        # branch-0 proj; hold the last 2 projs for the tail.
        micros = []
        for o in range(1, NDT):
            for ch in range(2):
                micros += qkv_half_micros(x2t_t, D, o, ch, kT2)
        for qi in range(4):
            micros += proj_micros(ot0, 0, qi)
        for qi in range(4, NQT):  # held for the tail: alternate pools
            micros += proj_micros(ot0, 0, qi,
                                  pool=(pp_s if qi % 2 == 0 else pp_x))
        ot1 = attention(kT2, vaug2, 1, micros, hold=16)

        # tail: the held branch-0 projs go first (they don't depend on
        # ot1's last normalize chain, covering its latency), then the
        # branch-1 projs, alternating psum pools for overlap.  pp_x
        # groups must run whole (bufs=1 ring) — never split a group
        # across another's.
        tail = list(micros)  # 4 held proj groups, 4 micros each
        order = [[tail.pop(0) for _ in range(4)],
                 [tail.pop(0) for _ in range(4)]]
        for qi in range(NQT):
            order.append(proj_micros(ot1, 1, qi,
                                     pool=(pp_s if qi % 2 == 0 else pp_x)))
            if qi < 2 and tail:
                order.append([tail.pop(0) for _ in range(4)])
        for grp in order:
            for m in grp:
                m()

    n = _split_multiwaits(nc)
    _built = (nc, n)
    return _built


def _host_prep(x, x2, qkv_w, proj_w, proj_b):
    """-> list of 8 per-core input maps; matmul operands in bfloat16,
    x^T/w tensors p-major ([128, i, cols]) for fat DMA descriptors."""
    import ml_dtypes
    bf16 = ml_dtypes.bfloat16
    b16 = lambda a: np.ascontiguousarray(np.asarray(a), dtype=bf16)

    def pmaj(m):  # [768, cols] -> [128, 6, cols]
        return np.ascontiguousarray(
            np.asarray(m).reshape(NDT, P, -1).transpose(1, 0, 2))

    xt = np.transpose(np.asarray(x), (0, 2, 1))      # [B, 768, 1024]
    x2t = np.transpose(np.asarray(x2), (0, 2, 1))
    wqk = b16(pmaj(np.asarray(qkv_w)[:2 * D].T))     # [128, 6, 1536]
    wv = b16(pmaj(np.asarray(qkv_w)[2 * D:].T))      # [128, 6, 768]
    wp = b16(pmaj(np.asarray(proj_w).T))             # [128, 6, 768]
    bias = np.broadcast_to(np.asarray(proj_b, dtype=np.float32),
                           (P, D)).copy()
    ones = np.ones((P, H, 1), dtype=bf16)
    maps = []
    for c in range(B):
        maps.append({
            "xt": b16(pmaj(xt[c])),
            "x2t": b16(pmaj(x2t[c])),
            "wqk": wqk, "wv": wv, "wp": wp, "bias": bias,
            "ones": ones,
        })
    return maps


def kernel(x, x2, qkv_w, proj_w, proj_b, trace=False, tmpdir=None):
    nc, _ = _build()
    from concourse.bass_utils import run_bass_kernel_spmd
    in_maps = _host_prep(x, x2, qkv_w, proj_w, proj_b)
    res = run_bass_kernel_spmd(nc, in_maps, list(range(B)), trace=trace,
                               tmpdir=tmpdir)
    kernel.last_exec_time_ns = res.exec_time_ns
    out = np.stack([res.results[c]["out"] for c in range(B)])  # [B,2,N,D]
    out1 = np.ascontiguousarray(out[:, 0])
    out2 = np.ascontiguousarray(out[:, 1])
    return (out1, out2)


kernel.last_exec_time_ns = None


# revision 37
# speedup vs baseline: 1.0455x; 1.0169x over previous
"""Two-branch attention kernel for Trainium2 (8 NeuronCores, batch-parallel).

out1 = proj(softmax(q k^T / 8) v),  out2 = proj(softmax(q k2^T / 8) v2)
with q,k,v from x and k2,v2 from x2 (q shared across branches).

Sharding: batch dim (8) -> one batch element per core. No collectives.

Design (per core, transpose-free attention, all-bf16 matmul operands):
  The attention inner loop is ScalarE-exp-paced (2.13us per kj vs 1.7us
  of S+AV matmul), and TRN2's HAM clock-gate re-throttles the PE to
  1.2GHz on every micro-idle, so the whole schedule is built around
  keeping the in-order TensorE stream gapless:
  * prefix: qT/kT group 0 + all v tiles (minimum to start attention);
  * every other QKV group (qT/kT 1-5, k2T, v2) and the projections are
    decomposed into ~0.5us micro-thunks (2-3 matmuls on a dedicated
    1-slot aux PSUM pool) popped between kj iterations so TensorE's
    per-kj work matches the exp pace;
  * AV is split into two 512-wide column passes (po tiles [65,512], one
    PSUM bank) - pass c0 trails S by 2 kj inside the kj loop, pass c1
    runs as a block after it - freeing the PSUM needed for aux;
  * softmax normalize: ones-row -> DRAM bounce -> [16,64] reshape (DVE
    reciprocal is per-lane-bound) -> reciprocal -> bounce out ->
    partition-broadcast read -> multiply straight out of PSUM;
  * wqk stays resident in SBUF so no weight DMAs run mid-attention.
"""
import sys
for _p in ('/opt/trn_rl_repo',):
    if _p not in sys.path:
        sys.path.insert(0, _p)

import numpy as np

# ----------------------------------------------------------------------------
MODE = 'bf16+microweave'  # informational only

B, N, D, H, HD = 8, 1024, 768, 12, 64
SCALE = HD ** -0.5
NDT = D // 128       # 6 dim tiles
NQT = N // 128       # 8 token tiles
P = 128

# ----------------------------------------------------------------------------
# workaround: walrus rejects >2 sem waits on one instruction; TileContext's
# tail drain carries one wait per active logical proc. Split them across
# single-wait SP nops and emit a bare drain.
def _install_tilefix():
    import bass_rust
    import concourse.tile as tile

    def _drain_and_barrier_split(self, tick_clock, wait_clock):
        gc = tick_clock.global_clock
        ticks = [gc[i] for i in range(27)]
        for i, t in enumerate(ticks):
            if t > 0:
                vc = bass_rust.VectorClock(
                    [t if j == i else 0 for j in range(len(ticks))])
                nop = self.nc.sync.nop()
                wait_clock.add_sem_waits(
                    nop.ins, bass_rust.ScopedClock({None: vc}))
        self.nc.sync.drain()
        self.nc.all_engine_barrier()
        assert self.sems is not None
        popped = self.nc._tile_sem_poison_stack.pop()
        assert popped is self._sem_poison
        self.nc.clear_and_free_semaphores(list(self.sems.allocated().values()))
        self.nc.all_engine_barrier()

    tile.TileContext._drain_and_barrier = _drain_and_barrier_split


def _split_multiwaits(nc, max_waits=1):
    """walrus codegen rejects instructions carrying more than `max_waits`
    sync waits; hoist the extras onto same-engine nops placed just before."""
    import bass_rust
    import concourse.mybir as mybir
    cnt = 0
    for bb in nc.main_func.blocks:
        insts = bb.instructions
        i = 0
        while i < len(insts):
            ins = insts[i]
            si = getattr(ins, 'sync_info', None)
            if si is not None and si.on_wait and len(si.on_wait) > max_waits:
                waits = list(si.on_wait)
                extras, keep = waits[:-max_waits], waits[-max_waits:]
                for w in extras:
                    nop = mybir.InstNoOp(name=f"I-swx{cnt}", ins=[], outs=[])
                    cnt += 1
                    nop.engine = ins.engine
                    nop.sync_info = bass_rust.SyncInfo(on_wait=[w],
                                                       on_update=[])
                    insts.insert(i, nop)
                    i += 1
                ins.sync_info = bass_rust.SyncInfo(
                    on_wait=keep, on_update=list(si.on_update))
            i += 1
    return cnt


_built = None


def _build():
    """Build the SPMD bass program once. Returns (nc, n_split_waits)."""
    global _built
    if _built is not None:
        return _built
    _install_tilefix()
    from contextlib import ExitStack
    import concourse.bass as bass
    import concourse.tile as tile
    from concourse import mybir

    dt = mybir.dt
    bdt = dt.bfloat16          # matmul operand dtype throughout

    nc = bass.Bass("TRN2", target_bir_lowering=False, debug=False,
                   num_devices=8)

    # DRAM I/O (per core); x/w tensors come p-major so the big loads are
    # 128 fat contiguous descriptors.
    xt_d = nc.dram_tensor("xt", [P, NDT, N], bdt, kind="ExternalInput")
    x2t_d = nc.dram_tensor("x2t", [P, NDT, N], bdt, kind="ExternalInput")
    wqk_d = nc.dram_tensor("wqk", [P, NDT, 2 * D], bdt,
                           kind="ExternalInput")
    wv_d = nc.dram_tensor("wv", [P, NDT, D], bdt, kind="ExternalInput")
    wp_d = nc.dram_tensor("wp", [P, NDT, D], bdt, kind="ExternalInput")
    bias_d = nc.dram_tensor("bias", [P, D], dt.float32, kind="ExternalInput")
    ones_d = nc.dram_tensor("ones", [P, H, 1], bdt, kind="ExternalInput")
    out_d = nc.dram_tensor("out", [2, N, D], dt.float32,
                           kind="ExternalOutput")

    AUG = HD + 1  # 65: head dim + ones column for row sums

    with tile.TileContext(nc) as tc, ExitStack() as top:
        # PSUM: 16KB/partition total.  8KB S ring + 4KB po ring + 3KB aux.
        pp_s = top.enter_context(tc.tile_pool(name="ps_s", bufs=2,
                                              space="PSUM"))
        pp_o = top.enter_context(tc.tile_pool(name="ps_o", bufs=2,
                                              space="PSUM"))
        pp_x = top.enter_context(tc.tile_pool(name="ps_x", bufs=1,
                                              space="PSUM"))
        dram_rb = top.enter_context(tc.tile_pool(name="dram_rb", bufs=2,
                                                 space="DRAM"))
        persist = top.enter_context(tc.tile_pool(name="persist", bufs=1))
        pool_kv = top.enter_context(tc.tile_pool(name="kv", bufs=1))

        qT = persist.tile([P, NDT, N], bdt, tag="qT")
        wqk_t = persist.tile([P, NDT, 2 * D], bdt, tag="wqk")
        for i in range(NDT):  # per-i chunks: parallel queues + pipelined
            nc.sync.dma_start(out=wqk_t[:, i, :], in_=wqk_d[:, i, :])
        wp_t = persist.tile([P, NDT, D], bdt, tag="wp")
        bias_t = persist.tile([P, D], dt.float32, tag="bias")
        nc.sync.dma_start(out=bias_t, in_=bias_d[:])

        kT = pool_kv.tile([P, NDT, N], bdt, tag="kT")
        kT2 = pool_kv.tile([P, NDT, N], bdt, tag="kT2")
        vaug = pool_kv.tile([P, NQT, H * AUG], bdt, tag="vaug")
        vaug2 = pool_kv.tile([P, NQT, H * AUG], bdt, tag="vaug2")
        xt_t = pool_kv.tile([P, NDT, N], bdt, tag="xt")
        x2t_t = pool_kv.tile([P, NDT, N], bdt, tag="x2t")
        wv_t = pool_kv.tile([P, NDT, D], bdt, tag="wv")

        def load_ones(vaug_t):
            for t in range(NQT):
                nc.sync.dma_start(
                    out=vaug_t[:, t, :].rearrange("p (h e) -> p h e",
                                                  e=AUG)[:, :, HD:AUG],
                    in_=ones_d[:])

        # ---------- macro helpers (prefix use, ScalarE evict) -----------
        def qkv_T_group(xt_tile, wcol0, o, dst_sb, evict):
            ps = pp_s.tile([P, N], dt.float32, tag="S")
            for i in range(NDT):
                for c in range(2):
                    nc.tensor.matmul(
                        ps[:, c * 512:(c + 1) * 512],
                        wqk_t[:, i, wcol0 + o * P: wcol0 + (o + 1) * P],
                        xt_tile[:, i, c * 512:(c + 1) * 512],
                        start=(i == 0), stop=(i == NDT - 1))
            evict(dst_sb[:, o, :], ps[:])

        def v_tile(xt_tile, vaug_t, t, evict):
            ps = pp_s.tile([P, N], dt.float32, tag="S")
            for i in range(NDT):
                for c0, cn in ((0, 512), (512, 256)):
                    nc.tensor.matmul(
                        ps[:, c0:c0 + cn],
                        xt_tile[:, i, t * P:(t + 1) * P],
                        wv_t[:, i, c0:c0 + cn],
                        start=(i == 0), stop=(i == NDT - 1))
            src = ps[:, 0:D].rearrange("p (h e) -> p h e", e=HD)
            dstv = vaug_t[:, t, :].rearrange("p (h e) -> p h e",
                                             e=AUG)[:, :, 0:HD]
            evict(dstv, src)

        # ---------- micro-thunks (attention-time fill, DVE evict) -------
        ev_vec = nc.vector.tensor_copy
        _auxbox = {}

        def qkv_half_micros(xt_tile, wcol0, o, ch, dst_sb):
            """3 micros: 6 accumulating 512-wide matmuls + DVE evict of
            one [128,512] column half of a q/k output group."""
            key = ('qk', id(xt_tile), wcol0, o, ch)

            def mm(i0, n, first, last):
                if first:
                    _auxbox[key] = pp_x.tile([P, 512], dt.float32,
                                             tag="aux", name=f"aux_qk")
                aux = _auxbox[key]
                for i in range(i0, i0 + n):
                    nc.tensor.matmul(
                        aux[:],
                        wqk_t[:, i, wcol0 + o * P: wcol0 + (o + 1) * P],
                        xt_tile[:, i, ch * 512:(ch + 1) * 512],
                        start=(i == 0), stop=(i == NDT - 1),
                        skip_group_check=True)
                if last:
                    ev_vec(dst_sb[:, o, ch * 512:(ch + 1) * 512], aux[:])
                    del _auxbox[key]

            return [lambda: mm(0, 2, True, False),
                    lambda: mm(2, 2, False, False),
                    lambda: mm(4, 2, False, True)]

        def v_tile_micros(xt_tile, vaug_t, t):
            """4 micros: 12 accumulating matmuls + DVE evict of one
            [128 tok, 768] v tile into the ones-augmented buffer."""
            key = ('v', id(xt_tile), t)
            steps = [(i, c0, cn) for i in range(NDT)
                     for c0, cn in ((0, 512), (512, 256))]

            def mm(s0, n, first, last):
                if first:
                    _auxbox[key] = pp_x.tile([P, D], dt.float32,
                                             tag="aux", name=f"aux_v")
                aux = _auxbox[key]
                for i, c0, cn in steps[s0:s0 + n]:
                    nc.tensor.matmul(
                        aux[:, c0:c0 + cn],
                        xt_tile[:, i, t * P:(t + 1) * P],
                        wv_t[:, i, c0:c0 + cn],
                        start=(i == 0), stop=(i == NDT - 1),
                        skip_group_check=True)
                if last:
                    src = aux[:, 0:D].rearrange("p (h e) -> p h e", e=HD)
                    dstv = vaug_t[:, t, :].rearrange(
                        "p (h e) -> p h e", e=AUG)[:, :, 0:HD]
                    ev_vec(dstv, src)
                    del _auxbox[key]

            return [lambda: mm(0, 3, True, False),
                    lambda: mm(3, 3, False, False),
                    lambda: mm(6, 3, False, False),
                    lambda: mm(9, 3, False, True)]

        pool_res = top.enter_context(tc.tile_pool(name="res", bufs=2))

        def proj_micros(ot_t, br, qi, pool=pp_x):
            """4 micros: 12 accumulating matmuls + bias add + DMA out of
            one [128 tok, 768] projection output tile."""
            key = ('p', br, qi)
            steps = [(g, c0, cn) for g in range(NDT)
                     for c0, cn in ((0, 512), (512, 256))]

            def mm(s0, n, first, last):
                if first:
                    _auxbox[key] = pool.tile([P, D], dt.float32,
                                             tag="aux" if pool is pp_x
                                             else "S", name=f"aux_p")
                aux = _auxbox[key]
                for g, c0, cn in steps[s0:s0 + n]:
                    nc.tensor.matmul(
                        aux[:, c0:c0 + cn],
                        ot_t[:, g, qi * P:(qi + 1) * P],
                        wp_t[:, g, c0:c0 + cn],
                        start=(g == 0), stop=(g == NDT - 1),
                        skip_group_check=True)
                if last:
                    res = pool_res.tile([P, D], dt.float32, tag="res")
                    nc.vector.tensor_add(res[:], aux[:], bias_t[:])
                    nc.sync.dma_start(
                        out=out_d[br, qi * P:(qi + 1) * P, :], in_=res[:])
                    del _auxbox[key]

            return [lambda: mm(0, 3, True, False),
                    lambda: mm(3, 3, False, False),
                    lambda: mm(6, 3, False, False),
                    lambda: mm(9, 3, False, True)]

        # ============ phase A prefix: minimum to start attention ========
        for i in range(NDT):
            nc.sync.dma_start(out=xt_t[:, i, :], in_=xt_d[:, i, :])
        for i in range(0, NDT, 2):
            nc.sync.dma_start(out=wv_t[:, i:i + 2, :],
                              in_=wv_d[:, i:i + 2, :])
        qkv_T_group(xt_t, 0, 0, qT, nc.scalar.copy)
        qkv_T_group(xt_t, D, 0, kT, nc.scalar.copy)
        load_ones(vaug)
        load_ones(vaug2)
        for t in range(NQT):
            v_tile(xt_t, vaug, t, nc.scalar.copy)
        # x2^T and W_proj aren't touched until well into attention;
        # emitting their loads here keeps them off the startup's
        # bandwidth-bound critical path.
        for i in range(0, NDT, 2):
            nc.sync.dma_start(out=x2t_t[:, i:i + 2, :],
                              in_=x2t_d[:, i:i + 2, :])
        for i in range(0, NDT, 3):
            nc.sync.dma_start(out=wp_t[:, i:i + 3, :],
                              in_=wp_d[:, i:i + 3, :])

        # ================= phase B: attention + proj ====================
        pool_pt = top.enter_context(tc.tile_pool(name="pt", bufs=2))
        pool_ot = top.enter_context(tc.tile_pool(name="ot", bufs=2))
        pool_sm = top.enter_context(tc.tile_pool(name="sm", bufs=2))
        pool_osb = top.enter_context(tc.tile_pool(name="osb", bufs=2))

        def normalize(po_c, ot, g, ch):
            """o^T[:, g, ch] /= rowsum via the ones rows of po_c[0..1].
            po is evicted to SBUF first (cheap) so the PSUM slot frees
            immediately; the reciprocal/broadcast DMA chain then runs
            fully async off the SBUF copy."""
            CW = 512
            osb = [pool_osb.tile([AUG, CW], bdt, tag="osb",
                                 name=f"osb{g}_{ch}_{hh}")
                   for hh in range(2)]
            for hh in range(2):
                # evict on ScalarE: 'copy' shares the exp ACT table (no
                # reload) and keeps the DVE queue short for aux handoffs
                nc.scalar.copy(osb[hh][:], po_c[hh][:])
            rb1 = dram_rb.tile([2, CW], bdt, tag="rb1")
            for hh in range(2):
                nc.sync.dma_start(out=rb1[hh:hh + 1, :],
                                  in_=osb[hh][HD:HD + 1, :])
            rgs = pool_sm.tile([16, HD], bdt, tag="rgs")
            nc.sync.dma_start(
                out=rgs[:], in_=rb1[:].rearrange("h (p e) -> (h p) e",
                                                 e=HD))
            rr = pool_sm.tile([16, HD], dt.float32, tag="rr")
            nc.vector.reciprocal(rr[:], rgs[:])
            rb2 = dram_rb.tile([2, CW], dt.float32, tag="rb2")
            nc.sync.dma_start(
                out=rb2[:].rearrange("h (p e) -> (h p) e", e=HD), in_=rr[:])
            for hh in range(2):
                rb = pool_sm.tile([HD, CW], dt.float32, tag="rb")
                nc.sync.dma_start(
                    out=rb[:], in_=rb2[hh, :].partition_broadcast(HD))
                nc.vector.tensor_mul(
                    ot[hh * HD:(hh + 1) * HD, g,
                       ch * 512:(ch + 1) * 512],
                    osb[hh][0:HD, :], rb[:])

        def attention(kT_t, vaug_t, br, micros, hold=0):
            """Exp-paced kj loop with AV pass c0 trailing S by 2 kj and
            micro-thunks popped per kj; AV pass c1 + second normalize run
            as the per-g tail block."""
            HQ = NQT // 2
            n0 = len(micros) - hold
            nslot = NDT * NQT
            done = [0]

            def pace(slot):
                want = n0 * (slot + 1) // nslot + min(4, slot + 1)
                while done[0] < want and len(micros) > hold:
                    micros.pop(0)()
                    done[0] += 1

            ot = pool_ot.tile([P, NDT, N], bdt, tag="ot")
            for g in range(NDT):
                po0 = [pp_o.tile([AUG, 512], dt.float32, tag="O",
                                 name=f"po0_{br}_{g}_{hh}")
                       for hh in range(2)]
                pth = {}

                def emit_av(po_c, ch, kj):
                    p = pth[kj // HQ]
                    for hh in range(2):
                        h = 2 * g + hh
                        nc.tensor.matmul(
                            po_c[hh][:],
                            vaug_t[:, kj, h * AUG:(h + 1) * AUG],
                            p[:, hh, kj % HQ, ch * 512:(ch + 1) * 512],
                            start=(kj == 0), stop=(kj == NQT - 1),
                            skip_group_check=True)

                for kj in range(NQT):
                    if kj % HQ == 0:
                        pth[kj // HQ] = pool_pt.tile(
                            [P, 2, HQ, N], bdt, tag="pt",
                            name=f"pth{br}_{g}_{kj // HQ}")
                    kjl = kj % HQ
                    pse = pp_s.tile([P, N], dt.float32, tag="S")
                    pso = pp_s.tile([P, N], dt.float32, tag="S")
                    for c in range(2):
                        nc.tensor.matmul(
                            pse[:, c * 512:(c + 1) * 512],
                            kT_t[0:HD, g, kj * P:(kj + 1) * P],
                            qT[0:HD, g, c * 512:(c + 1) * 512],
                            start=True, stop=True)
                        nc.tensor.matmul(
                            pso[:, c * 512:(c + 1) * 512],
                            kT_t[HD:P, g, kj * P:(kj + 1) * P],
                            qT[HD:P, g, c * 512:(c + 1) * 512],
                            start=True, stop=True)
                    nc.scalar.activation(
                        pth[kj // HQ][:, 0, kjl, :], pse[:],
                        mybir.ActivationFunctionType.Exp, scale=SCALE)
                    nc.scalar.activation(
                        pth[kj // HQ][:, 1, kjl, :], pso[:],
                        mybir.ActivationFunctionType.Exp, scale=SCALE)
                    if kj >= 2:
                        emit_av(po0, 0, kj - 2)
                    pace(g * NQT + kj)
                emit_av(po0, 0, NQT - 2)
                emit_av(po0, 0, NQT - 1)
                normalize(po0, ot, g, 0)
                po1 = [pp_o.tile([AUG, 512], dt.float32, tag="O",
                                 name=f"po1_{br}_{g}_{hh}")
                       for hh in range(2)]
                for kj in range(NQT):
                    emit_av(po1, 1, kj)
                normalize(po1, ot, g, 1)
            while len(micros) > hold:
                micros.pop(0)()
            return ot

        # branch 0: fill with qT/kT groups 1-5 (needed one g ahead),
        # x2's first k2T group, and all v2 tiles (needed by br1 start).
        micros = []
        for o in range(1, NDT):
            for ch in range(2):
                micros += qkv_half_micros(xt_t, 0, o, ch, qT)
            for ch in range(2):
                micros += qkv_half_micros(xt_t, D, o, ch, kT)
        for ch in range(2):
            micros += qkv_half_micros(x2t_t, D, 0, ch, kT2)
        for t in range(NQT - 2):
            micros += v_tile_micros(x2t_t, vaug2, t)
        ot0 = attention(kT, vaug, 0, micros)

        # branch 1: fill with the last two v2 tiles (needed by this
        # branch's own g0 AV, so they go first), x2's remaining k2T
        # groups (each one g ahead) and branch-0 proj; hold the last 2
        # projs for the tail.
        micros = v_tile_micros(x2t_t, vaug2, NQT - 2)
        for o in range(1, NDT):
            for ch in range(2):
                micros += qkv_half_micros(x2t_t, D, o, ch, kT2)
            if o == 1:
                micros += v_tile_micros(x2t_t, vaug2, NQT - 1)
        for qi in range(6):
            micros += proj_micros(ot0, 0, qi)
        for qi in range(6, NQT):  # held for the tail: alternate pools
            micros += proj_micros(ot0, 0, qi,
                                  pool=(pp_s if qi % 2 == 0 else pp_x))
        ot1 = attention(kT2, vaug2, 1, micros, hold=8)

        # tail: the held branch-0 projs go first (they don't depend on
        # ot1's last normalize chain, covering its latency), then the
        # branch-1 projs, alternating psum pools for overlap.  pp_x
        # groups must run whole (bufs=1 ring) — never split a group
        # across another's.
        tail = list(micros)  # 2 held proj groups, 4 micros each
        order = [[tail.pop(0) for _ in range(4)]]
        for qi in range(NQT):
            order.append(proj_micros(ot1, 1, qi,
                                     pool=(pp_s if qi % 2 == 0 else pp_x)))
            if qi == 0 and tail:
                order.append([tail.pop(0) for _ in range(4)])
        for grp in order:
            for m in grp:
                m()

    n = _split_multiwaits(nc)
    _built = (nc, n)
    return _built


def _host_prep(x, x2, qkv_w, proj_w, proj_b):
    """-> list of 8 per-core input maps; matmul operands in bfloat16,
    x^T/w tensors p-major ([128, i, cols]) for fat DMA descriptors."""
    import ml_dtypes
    bf16 = ml_dtypes.bfloat16
    b16 = lambda a: np.ascontiguousarray(np.asarray(a), dtype=bf16)

    def pmaj(m):  # [768, cols] -> [128, 6, cols]
        return np.ascontiguousarray(
            np.asarray(m).reshape(NDT, P, -1).transpose(1, 0, 2))

    xt = np.transpose(np.asarray(x), (0, 2, 1))      # [B, 768, 1024]
    x2t = np.transpose(np.asarray(x2), (0, 2, 1))
    wqk = b16(pmaj(np.asarray(qkv_w)[:2 * D].T))     # [128, 6, 1536]
    wv = b16(pmaj(np.asarray(qkv_w)[2 * D:].T))      # [128, 6, 768]
    wp = b16(pmaj(np.asarray(proj_w).T))             # [128, 6, 768]
    bias = np.broadcast_to(np.asarray(proj_b, dtype=np.float32),
                           (P, D)).copy()
    ones = np.ones((P, H, 1), dtype=bf16)
    maps = []
    for c in range(B):
        maps.append({
            "xt": b16(pmaj(xt[c])),
            "x2t": b16(pmaj(x2t[c])),
            "wqk": wqk, "wv": wv, "wp": wp, "bias": bias,
            "ones": ones,
        })
    return maps


def kernel(x, x2, qkv_w, proj_w, proj_b, trace=False, tmpdir=None):
    nc, _ = _build()
    from concourse.bass_utils import run_bass_kernel_spmd
    in_maps = _host_prep(x, x2, qkv_w, proj_w, proj_b)
    res = run_bass_kernel_spmd(nc, in_maps, list(range(B)), trace=trace,
                               tmpdir=tmpdir)
    kernel.last_exec_time_ns = res.exec_time_ns
    out = np.stack([res.results[c]["out"] for c in range(B)])  # [B,2,N,D]
    out1 = np.ascontiguousarray(out[:, 0])
    out2 = np.ascontiguousarray(out[:, 1])
    return (out1, out2)


kernel.last_exec_time_ns = None


# revision 45
# speedup vs baseline: 1.0480x; 1.0024x over previous
"""Two-branch attention kernel for Trainium2 (8 NeuronCores, batch-parallel).

out1 = proj(softmax(q k^T / 8) v),  out2 = proj(softmax(q k2^T / 8) v2)
with q,k,v from x and k2,v2 from x2 (q shared across branches).

Sharding: batch dim (8) -> one batch element per core. No collectives.

Design (per core, transpose-free attention, all-bf16 matmul operands):
  The attention inner loop is ScalarE-exp-paced (2.13us per kj vs 1.7us
  of S+AV matmul), and TRN2's HAM clock-gate re-throttles the PE to
  1.2GHz on every micro-idle, so the whole schedule is built around
  keeping the in-order TensorE stream gapless:
  * prefix: qT/kT group 0 + all v tiles (minimum to start attention);
  * every other QKV group (qT/kT 1-5, k2T, v2) and the projections are
    decomposed into ~0.5us micro-thunks (2-3 matmuls on a dedicated
    1-slot aux PSUM pool) popped between kj iterations so TensorE's
    per-kj work matches the exp pace;
  * AV is split into two 512-wide column passes (po tiles [65,512], one
    PSUM bank) - pass c0 trails S by 2 kj inside the kj loop, pass c1
    runs as a block after it - freeing the PSUM needed for aux;
  * softmax normalize: ones-row -> DRAM bounce -> [16,64] reshape (DVE
    reciprocal is per-lane-bound) -> reciprocal -> bounce out ->
    partition-broadcast read -> multiply straight out of PSUM;
  * wqk stays resident in SBUF so no weight DMAs run mid-attention.
"""
import sys
for _p in ('/opt/trn_rl_repo',):
    if _p not in sys.path:
        sys.path.insert(0, _p)

import numpy as np

# ----------------------------------------------------------------------------
MODE = 'bf16+microweave'  # informational only

B, N, D, H, HD = 8, 1024, 768, 12, 64
SCALE = HD ** -0.5
NDT = D // 128       # 6 dim tiles
NQT = N // 128       # 8 token tiles
P = 128

# ----------------------------------------------------------------------------
# workaround: walrus rejects >2 sem waits on one instruction; TileContext's
# tail drain carries one wait per active logical proc. Split them across
# single-wait SP nops and emit a bare drain.
def _install_tilefix():
    import bass_rust
    import concourse.tile as tile

    def _drain_and_barrier_split(self, tick_clock, wait_clock):
        gc = tick_clock.global_clock
        ticks = [gc[i] for i in range(27)]
        for i, t in enumerate(ticks):
            if t > 0:
                vc = bass_rust.VectorClock(
                    [t if j == i else 0 for j in range(len(ticks))])
                nop = self.nc.sync.nop()
                wait_clock.add_sem_waits(
                    nop.ins, bass_rust.ScopedClock({None: vc}))
        self.nc.sync.drain()
        self.nc.all_engine_barrier()
        assert self.sems is not None
        popped = self.nc._tile_sem_poison_stack.pop()
        assert popped is self._sem_poison
        self.nc.clear_and_free_semaphores(list(self.sems.allocated().values()))
        self.nc.all_engine_barrier()

    tile.TileContext._drain_and_barrier = _drain_and_barrier_split


def _split_multiwaits(nc, max_waits=1):
    """walrus codegen rejects instructions carrying more than `max_waits`
    sync waits; hoist the extras onto same-engine nops placed just before."""
    import bass_rust
    import concourse.mybir as mybir
    cnt = 0
    for bb in nc.main_func.blocks:
        insts = bb.instructions
        i = 0
        while i < len(insts):
            ins = insts[i]
            si = getattr(ins, 'sync_info', None)
            if si is not None and si.on_wait and len(si.on_wait) > max_waits:
                waits = list(si.on_wait)
                extras, keep = waits[:-max_waits], waits[-max_waits:]
                for w in extras:
                    nop = mybir.InstNoOp(name=f"I-swx{cnt}", ins=[], outs=[])
                    cnt += 1
                    nop.engine = ins.engine
                    nop.sync_info = bass_rust.SyncInfo(on_wait=[w],
                                                       on_update=[])
                    insts.insert(i, nop)
                    i += 1
                ins.sync_info = bass_rust.SyncInfo(
                    on_wait=keep, on_update=list(si.on_update))
            i += 1
    return cnt


_built = None


def _build():
    """Build the SPMD bass program once. Returns (nc, n_split_waits)."""
    global _built
    if _built is not None:
        return _built
    _install_tilefix()
    from contextlib import ExitStack
    import concourse.bass as bass
    import concourse.tile as tile
    from concourse import mybir

    dt = mybir.dt
    bdt = dt.bfloat16          # matmul operand dtype throughout

    nc = bass.Bass("TRN2", target_bir_lowering=False, debug=False,
                   num_devices=8)

    # DRAM I/O (per core); x/w tensors come p-major so the big loads are
    # 128 fat contiguous descriptors.
    xt_d = nc.dram_tensor("xt", [P, NDT, N], bdt, kind="ExternalInput")
    x2t_d = nc.dram_tensor("x2t", [P, NDT, N], bdt, kind="ExternalInput")
    # wqk in column-group-major layout [p, j, i, c]: group j covers output
    # columns j*128..j*128+128 (j 0-5 = q, 6-11 = k), so the startup only
    # waits on group 0/6 before the first prefix matmul.
    wqk_d = nc.dram_tensor("wqk", [P, 2 * NDT, NDT, P], bdt,
                           kind="ExternalInput")
    wv_d = nc.dram_tensor("wv", [P, NDT, D], bdt, kind="ExternalInput")
    wp_d = nc.dram_tensor("wp", [P, NDT, D], bdt, kind="ExternalInput")
    bias_d = nc.dram_tensor("bias", [P, D], dt.float32, kind="ExternalInput")
    ones_d = nc.dram_tensor("ones", [P, H, 1], bdt, kind="ExternalInput")
    out_d = nc.dram_tensor("out", [2, N, D], dt.float32,
                           kind="ExternalOutput")

    AUG = HD + 1  # 65: head dim + ones column for row sums

    with tile.TileContext(nc) as tc, ExitStack() as top:
        # PSUM: 16KB/partition total.  8KB S ring + 4KB po ring + 3KB aux.
        pp_s = top.enter_context(tc.tile_pool(name="ps_s", bufs=2,
                                              space="PSUM"))
        pp_o = top.enter_context(tc.tile_pool(name="ps_o", bufs=2,
                                              space="PSUM"))
        pp_x = top.enter_context(tc.tile_pool(name="ps_x", bufs=1,
                                              space="PSUM"))
        dram_rb = top.enter_context(tc.tile_pool(name="dram_rb", bufs=2,
                                                 space="DRAM"))
        persist = top.enter_context(tc.tile_pool(name="persist", bufs=1))
        pool_kv = top.enter_context(tc.tile_pool(name="kv", bufs=1))

        qT = persist.tile([P, NDT, N], bdt, tag="qT")
        wqk_t = persist.tile([P, 2 * NDT, NDT, P], bdt, tag="wqk")
        for j in [0, NDT] + [j for j in range(2 * NDT)
                             if j not in (0, NDT)]:
            nc.sync.dma_start(out=wqk_t[:, j, :, :], in_=wqk_d[:, j, :, :])
        wp_t = persist.tile([P, NDT, D], bdt, tag="wp")
        bias_t = persist.tile([P, D], dt.float32, tag="bias")
        nc.sync.dma_start(out=bias_t, in_=bias_d[:])

        kT = pool_kv.tile([P, NDT, N], bdt, tag="kT")
        kT2 = pool_kv.tile([P, NDT, N], bdt, tag="kT2")
        vaug = pool_kv.tile([P, NQT, H * AUG], bdt, tag="vaug")
        vaug2 = pool_kv.tile([P, NQT, H * AUG], bdt, tag="vaug2")
        xt_t = pool_kv.tile([P, NDT, N], bdt, tag="xt")
        x2t_t = pool_kv.tile([P, NDT, N], bdt, tag="x2t")
        wv_t = pool_kv.tile([P, NDT, D], bdt, tag="wv")

        def load_ones(vaug_t):
            for t in range(NQT):
                nc.sync.dma_start(
                    out=vaug_t[:, t, :].rearrange("p (h e) -> p h e",
                                                  e=AUG)[:, :, HD:AUG],
                    in_=ones_d[:])

        # ---------- macro helpers (prefix use, ScalarE evict) -----------
        def qkv_T_group(xt_tile, wcol0, o, dst_sb, evict):
            j = wcol0 // P + o
            ps = pp_s.tile([P, N], dt.float32, tag="S")
            for i in range(NDT):
                for c in range(2):
                    nc.tensor.matmul(
                        ps[:, c * 512:(c + 1) * 512],
                        wqk_t[:, j, i, :],
                        xt_tile[:, i, c * 512:(c + 1) * 512],
                        start=(i == 0), stop=(i == NDT - 1))
            evict(dst_sb[:, o, :], ps[:])

        def v_tile(xt_tile, vaug_t, t, evict):
            ps = pp_s.tile([P, N], dt.float32, tag="S")
            for i in range(NDT):
                for c0, cn in ((0, 512), (512, 256)):
                    nc.tensor.matmul(
                        ps[:, c0:c0 + cn],
                        xt_tile[:, i, t * P:(t + 1) * P],
                        wv_t[:, i, c0:c0 + cn],
                        start=(i == 0), stop=(i == NDT - 1))
            src = ps[:, 0:D].rearrange("p (h e) -> p h e", e=HD)
            dstv = vaug_t[:, t, :].rearrange("p (h e) -> p h e",
                                             e=AUG)[:, :, 0:HD]
            evict(dstv, src)

        # ---------- micro-thunks (attention-time fill, DVE evict) -------
        ev_vec = nc.vector.tensor_copy
        _auxbox = {}

        def qkv_half_micros(xt_tile, wcol0, o, ch, dst_sb):
            """3 micros: 6 accumulating 512-wide matmuls + DVE evict of
            one [128,512] column half of a q/k output group."""
            key = ('qk', id(xt_tile), wcol0, o, ch)
            j = wcol0 // P + o

            def mm(i0, n, first, last):
                if first:
                    _auxbox[key] = pp_x.tile([P, 512], dt.float32,
                                             tag="aux", name=f"aux_qk")
                aux = _auxbox[key]
                for i in range(i0, i0 + n):
                    nc.tensor.matmul(
                        aux[:],
                        wqk_t[:, j, i, :],
                        xt_tile[:, i, ch * 512:(ch + 1) * 512],
                        start=(i == 0), stop=(i == NDT - 1),
                        skip_group_check=True)
                if last:
                    ev_vec(dst_sb[:, o, ch * 512:(ch + 1) * 512], aux[:])
                    del _auxbox[key]

            return [lambda: mm(0, 2, True, False),
                    lambda: mm(2, 2, False, False),
                    lambda: mm(4, 2, False, True)]

        def v_tile_micros(xt_tile, vaug_t, t):
            """4 micros: 12 accumulating matmuls + DVE evict of one
            [128 tok, 768] v tile into the ones-augmented buffer."""
            key = ('v', id(xt_tile), t)
            steps = [(i, c0, cn) for i in range(NDT)
                     for c0, cn in ((0, 512), (512, 256))]

            def mm(s0, n, first, last):
                if first:
                    _auxbox[key] = pp_x.tile([P, D], dt.float32,
                                             tag="aux", name=f"aux_v")
                aux = _auxbox[key]
                for i, c0, cn in steps[s0:s0 + n]:
                    nc.tensor.matmul(
                        aux[:, c0:c0 + cn],
                        xt_tile[:, i, t * P:(t + 1) * P],
                        wv_t[:, i, c0:c0 + cn],
                        start=(i == 0), stop=(i == NDT - 1),
                        skip_group_check=True)
                if last:
                    src = aux[:, 0:D].rearrange("p (h e) -> p h e", e=HD)
                    dstv = vaug_t[:, t, :].rearrange(
                        "p (h e) -> p h e", e=AUG)[:, :, 0:HD]
                    ev_vec(dstv, src)
                    del _auxbox[key]

            return [lambda: mm(0, 3, True, False),
                    lambda: mm(3, 3, False, False),
                    lambda: mm(6, 3, False, False),
                    lambda: mm(9, 3, False, True)]

        pool_res = top.enter_context(tc.tile_pool(name="res", bufs=2))

        def proj_micros(ot_t, br, qi, pool=pp_x):
            """4 micros: 12 accumulating matmuls + bias add + DMA out of
            one [128 tok, 768] projection output tile."""
            key = ('p', br, qi)
            steps = [(g, c0, cn) for g in range(NDT)
                     for c0, cn in ((0, 512), (512, 256))]

            def mm(s0, n, first, last):
                if first:
                    _auxbox[key] = pool.tile([P, D], dt.float32,
                                             tag="aux" if pool is pp_x
                                             else "S", name=f"aux_p")
                aux = _auxbox[key]
                for g, c0, cn in steps[s0:s0 + n]:
                    nc.tensor.matmul(
                        aux[:, c0:c0 + cn],
                        ot_t[:, g, qi * P:(qi + 1) * P],
                        wp_t[:, g, c0:c0 + cn],
                        start=(g == 0), stop=(g == NDT - 1),
                        skip_group_check=True)
                if last:
                    res = pool_res.tile([P, D], dt.float32, tag="res")
                    nc.vector.tensor_add(res[:], aux[:], bias_t[:])
                    nc.sync.dma_start(
                        out=out_d[br, qi * P:(qi + 1) * P, :], in_=res[:])
                    del _auxbox[key]

            return [lambda: mm(0, 3, True, False),
                    lambda: mm(3, 3, False, False),
                    lambda: mm(6, 3, False, False),
                    lambda: mm(9, 3, False, True)]

        # ============ phase A prefix: minimum to start attention ========
        for i in range(NDT):
            nc.sync.dma_start(out=xt_t[:, i, :], in_=xt_d[:, i, :])
        for i in range(0, NDT, 2):
            nc.sync.dma_start(out=wv_t[:, i:i + 2, :],
                              in_=wv_d[:, i:i + 2, :])
        qkv_T_group(xt_t, 0, 0, qT, nc.scalar.copy)
        qkv_T_group(xt_t, D, 0, kT, nc.scalar.copy)
        load_ones(vaug)
        load_ones(vaug2)
        for t in range(NQT):
            v_tile(xt_t, vaug, t, nc.scalar.copy)
        # x2^T and W_proj aren't touched until well into attention;
        # emitting their loads here keeps them off the startup's
        # bandwidth-bound critical path.
        for i in range(0, NDT, 2):
            nc.sync.dma_start(out=x2t_t[:, i:i + 2, :],
                              in_=x2t_d[:, i:i + 2, :])
        for i in range(0, NDT, 3):
            nc.sync.dma_start(out=wp_t[:, i:i + 3, :],
                              in_=wp_d[:, i:i + 3, :])

        # ================= phase B: attention + proj ====================
        pool_pt = top.enter_context(tc.tile_pool(name="pt", bufs=2))
        pool_ot = top.enter_context(tc.tile_pool(name="ot", bufs=2))
        pool_sm = top.enter_context(tc.tile_pool(name="sm", bufs=2))
        pool_osb = top.enter_context(tc.tile_pool(name="osb", bufs=2))

        def normalize(po_c, ot, g, ch):
            """o^T[:, g, ch] /= rowsum via the ones rows of po_c[0..1].
            po is evicted to SBUF first (cheap) so the PSUM slot frees
            immediately; the reciprocal/broadcast DMA chain then runs
            fully async off the SBUF copy."""
            CW = 512
            osb = [pool_osb.tile([AUG, CW], bdt, tag="osb",
                                 name=f"osb{g}_{ch}_{hh}")
                   for hh in range(2)]
            for hh in range(2):
                # evict on ScalarE: 'copy' shares the exp ACT table (no
                # reload) and keeps the DVE queue short for aux handoffs
                nc.scalar.copy(osb[hh][:], po_c[hh][:])
            rb1 = dram_rb.tile([2, CW], bdt, tag="rb1")
            for hh in range(2):
                nc.sync.dma_start(out=rb1[hh:hh + 1, :],
                                  in_=osb[hh][HD:HD + 1, :])
            rgs = pool_sm.tile([16, HD], bdt, tag="rgs")
            nc.sync.dma_start(
                out=rgs[:], in_=rb1[:].rearrange("h (p e) -> (h p) e",
                                                 e=HD))
            rr = pool_sm.tile([16, HD], dt.float32, tag="rr")
            nc.vector.reciprocal(rr[:], rgs[:])
            rb2 = dram_rb.tile([2, CW], dt.float32, tag="rb2")
            nc.sync.dma_start(
                out=rb2[:].rearrange("h (p e) -> (h p) e", e=HD), in_=rr[:])
            for hh in range(2):
                rb = pool_sm.tile([HD, CW], dt.float32, tag="rb")
                nc.sync.dma_start(
                    out=rb[:], in_=rb2[hh, :].partition_broadcast(HD))
                nc.vector.tensor_mul(
                    ot[hh * HD:(hh + 1) * HD, g,
                       ch * 512:(ch + 1) * 512],
                    osb[hh][0:HD, :], rb[:])

        def attention(kT_t, vaug_t, br, micros, hold=0, deadlines=()):
            """Exp-paced kj loop with AV pass c0 trailing S by 2 kj and
            micro-thunks popped per kj; AV pass c1 + second normalize run
            as the per-g tail block.  `deadlines` = (slot, min_done)
            pairs forcing early pops for ordering-critical micros."""
            HQ = NQT // 2
            n0 = len(micros) - hold
            nslot = NDT * NQT
            done = [0]

            def pace(slot):
                want = n0 * (slot + 1) // nslot
                for s, m in deadlines:
                    if slot >= s:
                        want = max(want, m)
                while done[0] < want and len(micros) > hold:
                    micros.pop(0)()
                    done[0] += 1

            ot = pool_ot.tile([P, NDT, N], bdt, tag="ot")
            for g in range(NDT):
                po0 = [pp_o.tile([AUG, 512], dt.float32, tag="O",
                                 name=f"po0_{br}_{g}_{hh}")
                       for hh in range(2)]
                pth = {}

                def emit_av(po_c, ch, kj):
                    p = pth[kj // HQ]
                    for hh in range(2):
                        h = 2 * g + hh
                        nc.tensor.matmul(
                            po_c[hh][:],
                            vaug_t[:, kj, h * AUG:(h + 1) * AUG],
                            p[:, hh, kj % HQ, ch * 512:(ch + 1) * 512],
                            start=(kj == 0), stop=(kj == NQT - 1),
                            skip_group_check=True)

                for kj in range(NQT):
                    if kj % HQ == 0:
                        pth[kj // HQ] = pool_pt.tile(
                            [P, 2, HQ, N], bdt, tag="pt",
                            name=f"pth{br}_{g}_{kj // HQ}")
                    kjl = kj % HQ
                    pse = pp_s.tile([P, N], dt.float32, tag="S")
                    pso = pp_s.tile([P, N], dt.float32, tag="S")
                    for c in range(2):
                        nc.tensor.matmul(
                            pse[:, c * 512:(c + 1) * 512],
                            kT_t[0:HD, g, kj * P:(kj + 1) * P],
                            qT[0:HD, g, c * 512:(c + 1) * 512],
                            start=True, stop=True)
                        nc.tensor.matmul(
                            pso[:, c * 512:(c + 1) * 512],
                            kT_t[HD:P, g, kj * P:(kj + 1) * P],
                            qT[HD:P, g, c * 512:(c + 1) * 512],
                            start=True, stop=True)
                    nc.scalar.activation(
                        pth[kj // HQ][:, 0, kjl, :], pse[:],
                        mybir.ActivationFunctionType.Exp, scale=SCALE)
                    nc.scalar.activation(
                        pth[kj // HQ][:, 1, kjl, :], pso[:],
                        mybir.ActivationFunctionType.Exp, scale=SCALE)
                    if kj >= 2:
                        emit_av(po0, 0, kj - 2)
                    pace(g * NQT + kj)
                emit_av(po0, 0, NQT - 2)
                emit_av(po0, 0, NQT - 1)
                normalize(po0, ot, g, 0)
                po1 = [pp_o.tile([AUG, 512], dt.float32, tag="O",
                                 name=f"po1_{br}_{g}_{hh}")
                       for hh in range(2)]
                for kj in range(NQT):
                    emit_av(po1, 1, kj)
                normalize(po1, ot, g, 1)
            while len(micros) > hold:
                micros.pop(0)()
            return ot

        # branch 0: fill with qT/kT groups 1-5 (needed one g ahead),
        # x2's first k2T group, and all v2 tiles (needed by br1 start).
        micros = []
        for o in range(1, NDT):
            for ch in range(2):
                micros += qkv_half_micros(xt_t, 0, o, ch, qT)
            for ch in range(2):
                micros += qkv_half_micros(xt_t, D, o, ch, kT)
        for ch in range(2):
            micros += qkv_half_micros(x2t_t, D, 0, ch, kT2)
        for t in range(NQT - 2):
            micros += v_tile_micros(x2t_t, vaug2, t)
        # deadlines: qT/kT group o (micros 12o-12..12o) before g=o
        ot0 = attention(kT, vaug, 0, micros,
                        deadlines=[(4, 12), (12, 24), (20, 36), (28, 48),
                                   (34, 60)])

        # branch 1: fill with the last two v2 tiles (needed by this
        # branch's own g0 AV, so they go first), x2's remaining k2T
        # groups (each one g ahead) and branch-0 proj; hold the last 2
        # projs for the tail.
        micros = v_tile_micros(x2t_t, vaug2, NQT - 2)
        for o in range(1, NDT):
            for ch in range(2):
                micros += qkv_half_micros(x2t_t, D, o, ch, kT2)
            if o == 1:
                micros += v_tile_micros(x2t_t, vaug2, NQT - 1)
        for qi in range(6):
            micros += proj_micros(ot0, 0, qi)
        for qi in range(6, NQT):  # held for the tail: alternate pools
            micros += proj_micros(ot0, 0, qi,
                                  pool=(pp_s if qi % 2 == 0 else pp_x))
        # deadlines: v2(6)+k2T1 by g0-end/g1, v2(7) mid-g0, k2T o by g=o
        ot1 = attention(kT2, vaug2, 1, micros, hold=8,
                        deadlines=[(2, 5), (4, 10), (6, 14), (12, 20),
                                   (20, 26), (28, 32), (34, 38)])

        # tail: the held branch-0 projs go first (they don't depend on
        # ot1's last normalize chain, covering its latency), then the
        # branch-1 projs, alternating psum pools for overlap.  pp_x
        # groups must run whole (bufs=1 ring) — never split a group
        # across another's.
        tail = list(micros)  # 2 held proj groups, 4 micros each
        order = [[tail.pop(0) for _ in range(4)]]
        for qi in range(NQT):
            order.append(proj_micros(ot1, 1, qi,
                                     pool=(pp_s if qi % 2 == 0 else pp_x)))
            if qi == 0 and tail:
                order.append([tail.pop(0) for _ in range(4)])
        for grp in order:
            for m in grp:
                m()

    n = _split_multiwaits(nc)
    _built = (nc, n)
    return _built


def _host_prep(x, x2, qkv_w, proj_w, proj_b):
    """-> list of 8 per-core input maps; matmul operands in bfloat16,
    x^T/w tensors p-major ([128, i, cols]) for fat DMA descriptors."""
    import ml_dtypes
    bf16 = ml_dtypes.bfloat16
    b16 = lambda a: np.ascontiguousarray(np.asarray(a), dtype=bf16)

    def pmaj(m):  # [768, cols] -> [128, 6, cols]
        return np.ascontiguousarray(
            np.asarray(m).reshape(NDT, P, -1).transpose(1, 0, 2))

    xt = np.transpose(np.asarray(x), (0, 2, 1))      # [B, 768, 1024]
    x2t = np.transpose(np.asarray(x2), (0, 2, 1))
    # wqk: [768, 1536] -> p-major [128, 6(i), 1536] -> column-group-major
    # [128, 12(j), 6(i), 128]
    wqk = b16(np.ascontiguousarray(
        pmaj(np.asarray(qkv_w)[:2 * D].T)
        .reshape(P, NDT, 2 * NDT, P).transpose(0, 2, 1, 3)))
    wv = b16(pmaj(np.asarray(qkv_w)[2 * D:].T))      # [128, 6, 768]
    wp = b16(pmaj(np.asarray(proj_w).T))             # [128, 6, 768]
    bias = np.broadcast_to(np.asarray(proj_b, dtype=np.float32),
                           (P, D)).copy()
    ones = np.ones((P, H, 1), dtype=bf16)
    maps = []
    for c in range(B):
        maps.append({
            "xt": b16(pmaj(xt[c])),
            "x2t": b16(pmaj(x2t[c])),
            "wqk": wqk, "wv": wv, "wp": wp, "bias": bias,
            "ones": ones,
        })
    return maps


def kernel(x, x2, qkv_w, proj_w, proj_b, trace=False, tmpdir=None):
    nc, _ = _build()
    from concourse.bass_utils import run_bass_kernel_spmd
    in_maps = _host_prep(x, x2, qkv_w, proj_w, proj_b)
    res = run_bass_kernel_spmd(nc, in_maps, list(range(B)), trace=trace,
                               tmpdir=tmpdir)
    kernel.last_exec_time_ns = res.exec_time_ns
    out = np.stack([res.results[c]["out"] for c in range(B)])  # [B,2,N,D]
    out1 = np.ascontiguousarray(out[:, 0])
    out2 = np.ascontiguousarray(out[:, 1])
    return (out1, out2)


kernel.last_exec_time_ns = None


# revision 47
# speedup vs baseline: 1.0545x; 1.0062x over previous
"""Two-branch attention kernel for Trainium2 (8 NeuronCores, batch-parallel).

out1 = proj(softmax(q k^T / 8) v),  out2 = proj(softmax(q k2^T / 8) v2)
with q,k,v from x and k2,v2 from x2 (q shared across branches).

Sharding: batch dim (8) -> one batch element per core. No collectives.

Design (per core, transpose-free attention, all-bf16 matmul operands):
  The attention inner loop is ScalarE-exp-paced (2.13us per kj vs 1.7us
  of S+AV matmul), and TRN2's HAM clock-gate re-throttles the PE to
  1.2GHz on every micro-idle, so the whole schedule is built around
  keeping the in-order TensorE stream gapless:
  * prefix: qT/kT group 0 + all v tiles (minimum to start attention);
  * every other QKV group (qT/kT 1-5, k2T, v2) and the projections are
    decomposed into ~0.5us micro-thunks (2-3 matmuls on a dedicated
    1-slot aux PSUM pool) popped between kj iterations so TensorE's
    per-kj work matches the exp pace;
  * AV is split into two 512-wide column passes (po tiles [65,512], one
    PSUM bank) - pass c0 trails S by 2 kj inside the kj loop, pass c1
    runs as a block after it - freeing the PSUM needed for aux;
  * softmax normalize: ones-row -> DRAM bounce -> [16,64] reshape (DVE
    reciprocal is per-lane-bound) -> reciprocal -> bounce out ->
    partition-broadcast read -> multiply straight out of PSUM;
  * wqk stays resident in SBUF so no weight DMAs run mid-attention.
"""
import sys
for _p in ('/opt/trn_rl_repo',):
    if _p not in sys.path:
        sys.path.insert(0, _p)

import numpy as np

# ----------------------------------------------------------------------------
MODE = 'bf16+microweave'  # informational only

B, N, D, H, HD = 8, 1024, 768, 12, 64
SCALE = HD ** -0.5
NDT = D // 128       # 6 dim tiles
NQT = N // 128       # 8 token tiles
P = 128

# ----------------------------------------------------------------------------
# workaround: walrus rejects >2 sem waits on one instruction; TileContext's
# tail drain carries one wait per active logical proc. Split them across
# single-wait SP nops and emit a bare drain.
def _install_tilefix():
    import bass_rust
    import concourse.tile as tile

    def _drain_and_barrier_split(self, tick_clock, wait_clock):
        gc = tick_clock.global_clock
        ticks = [gc[i] for i in range(27)]
        for i, t in enumerate(ticks):
            if t > 0:
                vc = bass_rust.VectorClock(
                    [t if j == i else 0 for j in range(len(ticks))])
                nop = self.nc.sync.nop()
                wait_clock.add_sem_waits(
                    nop.ins, bass_rust.ScopedClock({None: vc}))
        self.nc.sync.drain()
        self.nc.all_engine_barrier()
        assert self.sems is not None
        popped = self.nc._tile_sem_poison_stack.pop()
        assert popped is self._sem_poison
        self.nc.clear_and_free_semaphores(list(self.sems.allocated().values()))
        self.nc.all_engine_barrier()

    tile.TileContext._drain_and_barrier = _drain_and_barrier_split


def _split_multiwaits(nc, max_waits=1):
    """walrus codegen rejects instructions carrying more than `max_waits`
    sync waits; hoist the extras onto same-engine nops placed just before."""
    import bass_rust
    import concourse.mybir as mybir
    cnt = 0
    for bb in nc.main_func.blocks:
        insts = bb.instructions
        i = 0
        while i < len(insts):
            ins = insts[i]
            si = getattr(ins, 'sync_info', None)
            if si is not None and si.on_wait and len(si.on_wait) > max_waits:
                waits = list(si.on_wait)
                extras, keep = waits[:-max_waits], waits[-max_waits:]
                for w in extras:
                    nop = mybir.InstNoOp(name=f"I-swx{cnt}", ins=[], outs=[])
                    cnt += 1
                    nop.engine = ins.engine
                    nop.sync_info = bass_rust.SyncInfo(on_wait=[w],
                                                       on_update=[])
                    insts.insert(i, nop)
                    i += 1
                ins.sync_info = bass_rust.SyncInfo(
                    on_wait=keep, on_update=list(si.on_update))
            i += 1
    return cnt


_built = None


def _build():
    """Build the SPMD bass program once. Returns (nc, n_split_waits)."""
    global _built
    if _built is not None:
        return _built
    _install_tilefix()
    from contextlib import ExitStack
    import concourse.bass as bass
    import concourse.tile as tile
    from concourse import mybir

    dt = mybir.dt
    bdt = dt.bfloat16          # matmul operand dtype throughout

    nc = bass.Bass("TRN2", target_bir_lowering=False, debug=False,
                   num_devices=8)

    # DRAM I/O (per core); x/w tensors come p-major so the big loads are
    # 128 fat contiguous descriptors.
    xt_d = nc.dram_tensor("xt", [P, NDT, N], bdt, kind="ExternalInput")
    x2t_d = nc.dram_tensor("x2t", [P, NDT, N], bdt, kind="ExternalInput")
    # wqk in column-group-major layout [p, j, i, c]: group j covers output
    # columns j*128..j*128+128 (j 0-5 = q, 6-11 = k), so the startup only
    # waits on group 0/6 before the first prefix matmul.
    wqk_d = nc.dram_tensor("wqk", [P, 2 * NDT, NDT, P], bdt,
                           kind="ExternalInput")
    wv_d = nc.dram_tensor("wv", [P, NDT, D], bdt, kind="ExternalInput")
    wp_d = nc.dram_tensor("wp", [P, NDT, D], bdt, kind="ExternalInput")
    bias_d = nc.dram_tensor("bias", [P, D], dt.float32, kind="ExternalInput")
    ones_d = nc.dram_tensor("ones", [P, H, 1], bdt, kind="ExternalInput")
    out_d = nc.dram_tensor("out", [2, N, D], dt.float32,
                           kind="ExternalOutput")

    AUG = HD + 1  # 65: head dim + ones column for row sums

    with tile.TileContext(nc) as tc, ExitStack() as top:
        # PSUM: 16KB/partition total.  8KB S ring + 4KB po ring + 3KB aux.
        pp_s = top.enter_context(tc.tile_pool(name="ps_s", bufs=2,
                                              space="PSUM"))
        pp_o = top.enter_context(tc.tile_pool(name="ps_o", bufs=2,
                                              space="PSUM"))
        pp_x = top.enter_context(tc.tile_pool(name="ps_x", bufs=1,
                                              space="PSUM"))
        dram_rb = top.enter_context(tc.tile_pool(name="dram_rb", bufs=2,
                                                 space="DRAM"))
        persist = top.enter_context(tc.tile_pool(name="persist", bufs=1))
        pool_kv = top.enter_context(tc.tile_pool(name="kv", bufs=1))

        qT = persist.tile([P, NDT, N], bdt, tag="qT")
        wqk_t = persist.tile([P, 2 * NDT, NDT, P], bdt, tag="wqk")
        for j in [0, NDT] + [j for j in range(2 * NDT)
                             if j not in (0, NDT)]:
            nc.sync.dma_start(out=wqk_t[:, j, :, :], in_=wqk_d[:, j, :, :])
        wp_t = persist.tile([P, NDT, D], bdt, tag="wp")
        bias_t = persist.tile([P, D], dt.float32, tag="bias")
        nc.sync.dma_start(out=bias_t, in_=bias_d[:])

        kT = pool_kv.tile([P, NDT, N], bdt, tag="kT")
        kT2 = pool_kv.tile([P, NDT, N], bdt, tag="kT2")
        vaug = pool_kv.tile([P, NQT, H * AUG], bdt, tag="vaug")
        vaug2 = pool_kv.tile([P, NQT, H * AUG], bdt, tag="vaug2")
        xt_t = pool_kv.tile([P, NDT, N], bdt, tag="xt")
        x2t_t = pool_kv.tile([P, NDT, N], bdt, tag="x2t")
        wv_t = pool_kv.tile([P, NDT, D], bdt, tag="wv")

        def load_ones(vaug_t):
            for t in range(NQT):
                nc.sync.dma_start(
                    out=vaug_t[:, t, :].rearrange("p (h e) -> p h e",
                                                  e=AUG)[:, :, HD:AUG],
                    in_=ones_d[:])

        # ---------- macro helpers (prefix use, ScalarE evict) -----------
        def qkv_T_group(xt_tile, wcol0, o, dst_sb, evict):
            j = wcol0 // P + o
            ps = pp_s.tile([P, N], dt.float32, tag="S")
            for i in range(NDT):
                for c in range(2):
                    nc.tensor.matmul(
                        ps[:, c * 512:(c + 1) * 512],
                        wqk_t[:, j, i, :],
                        xt_tile[:, i, c * 512:(c + 1) * 512],
                        start=(i == 0), stop=(i == NDT - 1))
            evict(dst_sb[:, o, :], ps[:])

        def v_tile(xt_tile, vaug_t, t, evict):
            ps = pp_s.tile([P, N], dt.float32, tag="S")
            for i in range(NDT):
                for c0, cn in ((0, 512), (512, 256)):
                    nc.tensor.matmul(
                        ps[:, c0:c0 + cn],
                        xt_tile[:, i, t * P:(t + 1) * P],
                        wv_t[:, i, c0:c0 + cn],
                        start=(i == 0), stop=(i == NDT - 1))
            src = ps[:, 0:D].rearrange("p (h e) -> p h e", e=HD)
            dstv = vaug_t[:, t, :].rearrange("p (h e) -> p h e",
                                             e=AUG)[:, :, 0:HD]
            evict(dstv, src)

        # ---------- micro-thunks (attention-time fill, DVE evict) -------
        ev_vec = nc.vector.tensor_copy
        _auxbox = {}

        def qkv_half_micros(xt_tile, wcol0, o, ch, dst_sb):
            """3 micros: 6 accumulating 512-wide matmuls + DVE evict of
            one [128,512] column half of a q/k output group."""
            key = ('qk', id(xt_tile), wcol0, o, ch)
            j = wcol0 // P + o

            def mm(i0, n, first, last):
                if first:
                    _auxbox[key] = pp_x.tile([P, 512], dt.float32,
                                             tag="aux", name=f"aux_qk")
                aux = _auxbox[key]
                for i in range(i0, i0 + n):
                    nc.tensor.matmul(
                        aux[:],
                        wqk_t[:, j, i, :],
                        xt_tile[:, i, ch * 512:(ch + 1) * 512],
                        start=(i == 0), stop=(i == NDT - 1),
                        skip_group_check=True)
                if last:
                    ev_vec(dst_sb[:, o, ch * 512:(ch + 1) * 512], aux[:])
                    del _auxbox[key]

            return [lambda: mm(0, 2, True, False),
                    lambda: mm(2, 2, False, False),
                    lambda: mm(4, 2, False, True)]

        def v_tile_micros(xt_tile, vaug_t, t):
            """4 micros: 12 accumulating matmuls + DVE evict of one
            [128 tok, 768] v tile into the ones-augmented buffer."""
            key = ('v', id(xt_tile), t)
            steps = [(i, c0, cn) for i in range(NDT)
                     for c0, cn in ((0, 512), (512, 256))]

            def mm(s0, n, first, last):
                if first:
                    _auxbox[key] = pp_x.tile([P, D], dt.float32,
                                             tag="aux", name=f"aux_v")
                aux = _auxbox[key]
                for i, c0, cn in steps[s0:s0 + n]:
                    nc.tensor.matmul(
                        aux[:, c0:c0 + cn],
                        xt_tile[:, i, t * P:(t + 1) * P],
                        wv_t[:, i, c0:c0 + cn],
                        start=(i == 0), stop=(i == NDT - 1),
                        skip_group_check=True)
                if last:
                    src = aux[:, 0:D].rearrange("p (h e) -> p h e", e=HD)
                    dstv = vaug_t[:, t, :].rearrange(
                        "p (h e) -> p h e", e=AUG)[:, :, 0:HD]
                    ev_vec(dstv, src)
                    del _auxbox[key]

            return [lambda: mm(0, 3, True, False),
                    lambda: mm(3, 3, False, False),
                    lambda: mm(6, 3, False, False),
                    lambda: mm(9, 3, False, True)]

        pool_res = top.enter_context(tc.tile_pool(name="res", bufs=2))

        def proj_micros(ot_t, br, qi, pool=pp_x):
            """4 micros: 12 accumulating matmuls + bias add + DMA out of
            one [128 tok, 768] projection output tile."""
            key = ('p', br, qi)
            steps = [(g, c0, cn) for g in range(NDT)
                     for c0, cn in ((0, 512), (512, 256))]

            def mm(s0, n, first, last):
                if first:
                    _auxbox[key] = pool.tile([P, D], dt.float32,
                                             tag="aux" if pool is pp_x
                                             else "S", name=f"aux_p")
                aux = _auxbox[key]
                for g, c0, cn in steps[s0:s0 + n]:
                    nc.tensor.matmul(
                        aux[:, c0:c0 + cn],
                        ot_t[:, g, qi * P:(qi + 1) * P],
                        wp_t[:, g, c0:c0 + cn],
                        start=(g == 0), stop=(g == NDT - 1),
                        skip_group_check=True)
                if last:
                    res = pool_res.tile([P, D], dt.float32, tag="res")
                    nc.vector.tensor_add(res[:], aux[:], bias_t[:])
                    nc.sync.dma_start(
                        out=out_d[br, qi * P:(qi + 1) * P, :], in_=res[:])
                    del _auxbox[key]

            return [lambda: mm(0, 3, True, False),
                    lambda: mm(3, 3, False, False),
                    lambda: mm(6, 3, False, False),
                    lambda: mm(9, 3, False, True)]

        # ============ phase A prefix: minimum to start attention ========
        for i in range(NDT):
            nc.sync.dma_start(out=xt_t[:, i, :], in_=xt_d[:, i, :])
        for i in range(0, NDT, 2):
            nc.sync.dma_start(out=wv_t[:, i:i + 2, :],
                              in_=wv_d[:, i:i + 2, :])
        qkv_T_group(xt_t, 0, 0, qT, nc.scalar.copy)
        qkv_T_group(xt_t, D, 0, kT, nc.scalar.copy)
        load_ones(vaug)
        load_ones(vaug2)
        for t in range(NQT):
            v_tile(xt_t, vaug, t, nc.scalar.copy)
        # x2^T and W_proj aren't touched until well into attention; a
        # dummy write sourced from v-tile-0's output forces their loads
        # (WAW-ordered behind it) off the startup's bandwidth-bound
        # critical path — queue position alone doesn't defer a DMA.
        for i in range(0, NDT, 2):
            nc.vector.tensor_copy(x2t_t[0:1, i, 0:1], vaug[0:1, 0, 0:1])
        for i in range(0, NDT, 3):
            nc.vector.tensor_copy(wp_t[0:1, i, 0:1], vaug[0:1, 0, 0:1])
        for i in range(0, NDT, 2):
            nc.sync.dma_start(out=x2t_t[:, i:i + 2, :],
                              in_=x2t_d[:, i:i + 2, :])
        for i in range(0, NDT, 3):
            nc.sync.dma_start(out=wp_t[:, i:i + 3, :],
                              in_=wp_d[:, i:i + 3, :])

        # ================= phase B: attention + proj ====================
        pool_pt = top.enter_context(tc.tile_pool(name="pt", bufs=2))
        pool_ot = top.enter_context(tc.tile_pool(name="ot", bufs=2))
        pool_sm = top.enter_context(tc.tile_pool(name="sm", bufs=2))
        pool_osb = top.enter_context(tc.tile_pool(name="osb", bufs=2))

        def normalize(po_c, ot, g, ch):
            """o^T[:, g, ch] /= rowsum via the ones rows of po_c[0..1].
            po is evicted to SBUF first (cheap) so the PSUM slot frees
            immediately; the reciprocal/broadcast DMA chain then runs
            fully async off the SBUF copy."""
            CW = 512
            osb = [pool_osb.tile([AUG, CW], bdt, tag="osb",
                                 name=f"osb{g}_{ch}_{hh}")
                   for hh in range(2)]
            for hh in range(2):
                # evict on ScalarE: 'copy' shares the exp ACT table (no
                # reload) and keeps the DVE queue short for aux handoffs
                nc.scalar.copy(osb[hh][:], po_c[hh][:])
            rb1 = dram_rb.tile([2, CW], bdt, tag="rb1")
            for hh in range(2):
                nc.sync.dma_start(out=rb1[hh:hh + 1, :],
                                  in_=osb[hh][HD:HD + 1, :])
            rgs = pool_sm.tile([16, HD], bdt, tag="rgs")
            nc.sync.dma_start(
                out=rgs[:], in_=rb1[:].rearrange("h (p e) -> (h p) e",
                                                 e=HD))
            rr = pool_sm.tile([16, HD], dt.float32, tag="rr")
            nc.vector.reciprocal(rr[:], rgs[:])
            rb2 = dram_rb.tile([2, CW], dt.float32, tag="rb2")
            nc.sync.dma_start(
                out=rb2[:].rearrange("h (p e) -> (h p) e", e=HD), in_=rr[:])
            for hh in range(2):
                rb = pool_sm.tile([HD, CW], dt.float32, tag="rb")
                nc.sync.dma_start(
                    out=rb[:], in_=rb2[hh, :].partition_broadcast(HD))
                nc.vector.tensor_mul(
                    ot[hh * HD:(hh + 1) * HD, g,
                       ch * 512:(ch + 1) * 512],
                    osb[hh][0:HD, :], rb[:])

        def attention(kT_t, vaug_t, br, micros, hold=0, deadlines=()):
            """Exp-paced kj loop with AV pass c0 trailing S by 2 kj and
            micro-thunks popped per kj; AV pass c1 + second normalize run
            as the per-g tail block.  `deadlines` = (slot, min_done)
            pairs forcing early pops for ordering-critical micros."""
            HQ = NQT // 2
            n0 = len(micros) - hold
            nslot = NDT * NQT
            done = [0]

            def pace(slot):
                want = n0 * (slot + 1) // nslot
                for s, m in deadlines:
                    if slot >= s:
                        want = max(want, m)
                while done[0] < want and len(micros) > hold:
                    micros.pop(0)()
                    done[0] += 1

            ot = pool_ot.tile([P, NDT, N], bdt, tag="ot")
            for g in range(NDT):
                po0 = [pp_o.tile([AUG, 512], dt.float32, tag="O",
                                 name=f"po0_{br}_{g}_{hh}")
                       for hh in range(2)]
                pth = {}

                def emit_av(po_c, ch, kj):
                    p = pth[kj // HQ]
                    for hh in range(2):
                        h = 2 * g + hh
                        nc.tensor.matmul(
                            po_c[hh][:],
                            vaug_t[:, kj, h * AUG:(h + 1) * AUG],
                            p[:, hh, kj % HQ, ch * 512:(ch + 1) * 512],
                            start=(kj == 0), stop=(kj == NQT - 1),
                            skip_group_check=True)

                for kj in range(NQT):
                    if kj % HQ == 0:
                        pth[kj // HQ] = pool_pt.tile(
                            [P, 2, HQ, N], bdt, tag="pt",
                            name=f"pth{br}_{g}_{kj // HQ}")
                    kjl = kj % HQ
                    pse = pp_s.tile([P, N], dt.float32, tag="S")
                    pso = pp_s.tile([P, N], dt.float32, tag="S")
                    for c in range(2):
                        nc.tensor.matmul(
                            pse[:, c * 512:(c + 1) * 512],
                            kT_t[0:HD, g, kj * P:(kj + 1) * P],
                            qT[0:HD, g, c * 512:(c + 1) * 512],
                            start=True, stop=True)
                        nc.tensor.matmul(
                            pso[:, c * 512:(c + 1) * 512],
                            kT_t[HD:P, g, kj * P:(kj + 1) * P],
                            qT[HD:P, g, c * 512:(c + 1) * 512],
                            start=True, stop=True)
                    nc.scalar.activation(
                        pth[kj // HQ][:, 0, kjl, :], pse[:],
                        mybir.ActivationFunctionType.Exp, scale=SCALE)
                    nc.scalar.activation(
                        pth[kj // HQ][:, 1, kjl, :], pso[:],
                        mybir.ActivationFunctionType.Exp, scale=SCALE)
                    if kj >= 2:
                        emit_av(po0, 0, kj - 2)
                    pace(g * NQT + kj)
                emit_av(po0, 0, NQT - 2)
                emit_av(po0, 0, NQT - 1)
                normalize(po0, ot, g, 0)
                po1 = [pp_o.tile([AUG, 512], dt.float32, tag="O",
                                 name=f"po1_{br}_{g}_{hh}")
                       for hh in range(2)]
                for kj in range(NQT):
                    emit_av(po1, 1, kj)
                normalize(po1, ot, g, 1)
            while len(micros) > hold:
                micros.pop(0)()
            return ot

        # branch 0: fill with qT/kT groups 1-5 (needed one g ahead),
        # x2's first k2T group, and all v2 tiles (needed by br1 start).
        micros = []
        for o in range(1, NDT):
            for ch in range(2):
                micros += qkv_half_micros(xt_t, 0, o, ch, qT)
            for ch in range(2):
                micros += qkv_half_micros(xt_t, D, o, ch, kT)
        for ch in range(2):
            micros += qkv_half_micros(x2t_t, D, 0, ch, kT2)
        for t in range(NQT - 2):
            micros += v_tile_micros(x2t_t, vaug2, t)
        # deadlines: qT/kT group o (micros 12o-12..12o) before g=o
        ot0 = attention(kT, vaug, 0, micros,
                        deadlines=[(4, 12), (12, 24), (20, 36), (28, 48),
                                   (34, 60)])

        # branch 1: fill with the last two v2 tiles (needed by this
        # branch's own g0 AV, so they go first), x2's remaining k2T
        # groups (each one g ahead) and branch-0 proj; hold the last 2
        # projs for the tail.
        micros = v_tile_micros(x2t_t, vaug2, NQT - 2)
        for o in range(1, NDT):
            for ch in range(2):
                micros += qkv_half_micros(x2t_t, D, o, ch, kT2)
            if o == 1:
                micros += v_tile_micros(x2t_t, vaug2, NQT - 1)
        for qi in range(6):
            micros += proj_micros(ot0, 0, qi)
        for qi in range(6, NQT):  # held for the tail: alternate pools
            micros += proj_micros(ot0, 0, qi,
                                  pool=(pp_s if qi % 2 == 0 else pp_x))
        # deadlines: v2(6)+k2T1 by g0-end/g1, v2(7) mid-g0, k2T o by g=o
        ot1 = attention(kT2, vaug2, 1, micros, hold=8,
                        deadlines=[(2, 5), (4, 10), (6, 14), (12, 20),
                                   (20, 26), (28, 32), (34, 38)])

        # tail: the held branch-0 projs go first (they don't depend on
        # ot1's last normalize chain, covering its latency), then the
        # branch-1 projs, alternating psum pools for overlap.  pp_x
        # groups must run whole (bufs=1 ring) — never split a group
        # across another's.
        tail = list(micros)  # 2 held proj groups, 4 micros each
        order = [[tail.pop(0) for _ in range(4)]]
        for qi in range(NQT):
            order.append(proj_micros(ot1, 1, qi,
                                     pool=(pp_s if qi % 2 == 0 else pp_x)))
            if qi == 0 and tail:
                order.append([tail.pop(0) for _ in range(4)])
        for grp in order:
            for m in grp:
                m()

    n = _split_multiwaits(nc)
    _built = (nc, n)
    return _built


def _host_prep(x, x2, qkv_w, proj_w, proj_b):
    """-> list of 8 per-core input maps; matmul operands in bfloat16,
    x^T/w tensors p-major ([128, i, cols]) for fat DMA descriptors."""
    import ml_dtypes
    bf16 = ml_dtypes.bfloat16
    b16 = lambda a: np.ascontiguousarray(np.asarray(a), dtype=bf16)

    def pmaj(m):  # [768, cols] -> [128, 6, cols]
        return np.ascontiguousarray(
            np.asarray(m).reshape(NDT, P, -1).transpose(1, 0, 2))

    xt = np.transpose(np.asarray(x), (0, 2, 1))      # [B, 768, 1024]
    x2t = np.transpose(np.asarray(x2), (0, 2, 1))
    # wqk: [768, 1536] -> p-major [128, 6(i), 1536] -> column-group-major
    # [128, 12(j), 6(i), 128]
    wqk = b16(np.ascontiguousarray(
        pmaj(np.asarray(qkv_w)[:2 * D].T)
        .reshape(P, NDT, 2 * NDT, P).transpose(0, 2, 1, 3)))
    wv = b16(pmaj(np.asarray(qkv_w)[2 * D:].T))      # [128, 6, 768]
    wp = b16(pmaj(np.asarray(proj_w).T))             # [128, 6, 768]
    bias = np.broadcast_to(np.asarray(proj_b, dtype=np.float32),
                           (P, D)).copy()
    ones = np.ones((P, H, 1), dtype=bf16)
    maps = []
    for c in range(B):
        maps.append({
            "xt": b16(pmaj(xt[c])),
            "x2t": b16(pmaj(x2t[c])),
            "wqk": wqk, "wv": wv, "wp": wp, "bias": bias,
            "ones": ones,
        })
    return maps


def kernel(x, x2, qkv_w, proj_w, proj_b, trace=False, tmpdir=None):
    nc, _ = _build()
    from concourse.bass_utils import run_bass_kernel_spmd
    in_maps = _host_prep(x, x2, qkv_w, proj_w, proj_b)
    res = run_bass_kernel_spmd(nc, in_maps, list(range(B)), trace=trace,
                               tmpdir=tmpdir)
    kernel.last_exec_time_ns = res.exec_time_ns
    out = np.stack([res.results[c]["out"] for c in range(B)])  # [B,2,N,D]
    out1 = np.ascontiguousarray(out[:, 0])
    out2 = np.ascontiguousarray(out[:, 1])
    return (out1, out2)


kernel.last_exec_time_ns = None
